# revision 1
# baseline (speedup 1.0000x reference)
"""DeepseekV3 MLA attention kernel for 8 Trainium2 NeuronCores.

Sharding (tensor-parallel over heads + data-parallel over tokens):
  - Stage A (per core, its 256-token slice): latent = hidden @ Wkva first,
    rmsnorm + rope k_pe, AllGather it (AG_kv) while q_a = hidden @ Wqa still
    computes; q_a_n then AllGathered in two halves.  All feature-major.
  - Stage B3 (overlaps AG_qa): k_nope / v = Wkvb_c.T @ kv_a_n for this core's
    4 heads, v produced token-major directly.
  - Stage B1/B2: q = Wqb_c.T @ q_a_n, rope q_pe in place (swap via PE
    permutation matmul).
  - Stage B4: causal attention per head, scores kept transposed (k on
    partitions); softmax denominator accumulated on the PE via a ones-column
    matmul per k-chunk, reciprocal on a [1 x 512] row, broadcast back to 128
    partitions with a ones-row matmul; no max-subtraction (scores verified
    small).  After each head: that head-slot's AllGather (bf16) runs while
    later heads compute, and the corresponding slice of the output projection
    accumulates into SBUF.
  - Stage C (interleaved): out_c = sum_j Wo_j.T @ attn_j with Wo in bf16,
    rows host-permuted to the per-head-slot gather order.

All wire payloads (AllGathers) and B-stage weights/activations are bf16; the
rms scales travel as bf16 hi/lo row pairs to keep full fp32 accuracy.  PSUM
accumulation stays fp32 throughout.
"""

import math
import sys
import types

import numpy as np

# ---------------------------------------------------------------- constants
H = 32
D_NOPE = 128
D_ROPE = 64
D_QK = 192
D_V = 128
KV_LORA = 512
EPS = 1e-6
ROPE_THETA = 10000.0
FACTOR = 40.0
BETA_FAST, BETA_SLOW = 32.0, 1.0
ORIG_MAX_POS = 4096
MSCALE_ALL_DIM = 1.0

T = 2048
HID = 4096
QA = 1536  # q lora rank
NCORES = 8
HL = H // NCORES          # 4 heads per core
TSH = T // NCORES         # 256 tokens per core
QB_N = HL * D_QK          # 768 q columns per core
KB_N = HL * (D_NOPE + D_V)  # 1024 kv columns per core
WO_N = HID // NCORES      # 512 output columns per core
KVB = KV_LORA + D_ROPE    # 576
QAH = QA // 2             # 768, AG_qa half

_CACHE = {}


def _yarn_inv_freq():
    dim = D_ROPE
    pos_freqs = ROPE_THETA ** (np.arange(0, dim, 2, dtype=np.float64) / dim)
    inv_extra = 1.0 / pos_freqs
    inv_inter = 1.0 / (FACTOR * pos_freqs)

    def corr_dim(n_rot):
        return dim * math.log(ORIG_MAX_POS / (n_rot * 2 * math.pi)) / (2 * math.log(ROPE_THETA))

    low = max(math.floor(corr_dim(BETA_FAST)), 0)
    high = min(math.ceil(corr_dim(BETA_SLOW)), dim - 1)
    ramp = np.clip((np.arange(dim // 2, dtype=np.float64) - low) / max(high - low, 1e-3), 0, 1)
    inv_freq_mask = 1.0 - ramp
    inv_freq = inv_inter * (1 - inv_freq_mask) + inv_extra * inv_freq_mask
    return inv_freq.astype(np.float32)


def _install_ntff_hook():
    """Shim antenv.axon_hooks so run_bass_kernel_spmd(trace=True) can profile."""
    if "antenv.axon_hooks" in sys.modules:
        return
    mod = types.ModuleType("antenv.axon_hooks")
    mod._hook = None

    def set_axon_ntff_profile_hook(h):
        mod._hook = h

    def get_axon_ntff_profile_hook():
        return mod._hook

    mod.set_axon_ntff_profile_hook = set_axon_ntff_profile_hook
    mod.get_axon_ntff_profile_hook = get_axon_ntff_profile_hook
    sys.modules["antenv.axon_hooks"] = mod
    try:
        import antenv

        antenv.axon_hooks = mod
        from trn_agent_boot.trn_boot import _ntff_profile_via_ctypes

        hook = _ntff_profile_via_ctypes("/opt/axon/libaxon_pjrt.so")
        if hook is not None:
            set_axon_ntff_profile_hook(hook)
    except Exception:
        pass


# ---------------------------------------------------------------- program
def _build_program():
    if "nc" in _CACHE:
        return _CACHE["nc"]

    import concourse.bacc as bacc
    import concourse.tile as tile
    from concourse import mybir

    R = mybir.dt.float32r
    F = mybir.dt.float32
    B16 = mybir.dt.bfloat16
    AF = mybir.ActivationFunctionType

    nc = bacc.Bacc("TRN2", target_bir_lowering=False, debug=False, num_devices=NCORES)

    # ------------- DRAM I/O (per-core values fed via in_maps)
    hT = nc.dram_tensor("hT", [HID, TSH], B16, kind="ExternalInput")
    wqa = nc.dram_tensor("wqa", [HID, QA], B16, kind="ExternalInput")
    wkva = nc.dram_tensor("wkva", [HID, KVB], B16, kind="ExternalInput")
    wqb = nc.dram_tensor("wqb", [QA, QB_N], B16, kind="ExternalInput")
    wkvb = nc.dram_tensor("wkvb", [KV_LORA, KB_N], B16, kind="ExternalInput")
    wo = nc.dram_tensor("wo", [H * D_V, WO_N], B16, kind="ExternalInput")
    cca = nc.dram_tensor("cca", [D_ROPE // 2, TSH], F, kind="ExternalInput")
    ssa = nc.dram_tensor("ssa", [D_ROPE // 2, TSH], F, kind="ExternalInput")
    ccq = nc.dram_tensor("ccq", [128, T], F, kind="ExternalInput")
    ssq = nc.dram_tensor("ssq", [128, T], F, kind="ExternalInput")
    psw = nc.dram_tensor("psw", [128, 128], B16, kind="ExternalInput")
    maskd = nc.dram_tensor("maskd", [128, 4 * 512], B16, kind="ExternalInput")
    onesd = nc.dram_tensor("onesd", [128, 128], B16, kind="ExternalInput")
    out = nc.dram_tensor("out", [WO_N, T], F, kind="ExternalOutput")

    NKH = HID // 128   # 32 hid chunks
    NKQ = QA // 128    # 12 q-lora chunks
    NKV = KV_LORA // 128  # 4 kv-lora chunks
    NB = 4             # token blocks of 512
    TB = 512
    RG = [list(range(NCORES))]

    with tile.TileContext(nc) as tc:
        with (
            tc.tile_pool(name="consts", bufs=1) as consts,
            tc.tile_pool(name="dram", bufs=1, space="DRAM") as dram,
        ):
            ones_sb = consts.tile([128, 128], B16)
            nc.sync.dma_start(out=ones_sb[:], in_=onesd[:])
            ones_rf = consts.tile([1, 128], R)
            ones_cf = consts.tile([128, 1], R)
            with nc.allow_low_precision(reason="exact ones, fp32r for PE broadcasts"):
                nc.vector.tensor_copy(ones_rf[:], ones_sb[0:1, :])
                nc.vector.tensor_copy(ones_cf[:], ones_sb[:, 0:1])
            eps_sb = consts.tile([1, 1], F)
            nc.vector.memset(eps_sb[:], EPS)

            KVB2 = KVB + 2   # 578 rows per rank in ag_kv (576 + scale hi/lo)
            QAH2 = QAH + 2   # 770 rows per rank in ag_qa[1] (768 + scale hi/lo)
            ag_kv_in = dram.tile([KVB2, TSH], B16)
            ag_kv_out = dram.tile([NCORES * KVB2, TSH], B16, addr_space="Shared")
            ag_qa_in = [dram.tile([QAH + 2 * g, TSH], B16, name=f"agqi{g}", tag=f"agqi{g}")
                        for g in range(2)]
            ag_qa_out = [dram.tile([NCORES * (QAH + 2 * g), TSH], B16, name=f"agqo{g}",
                                   tag=f"agqo{g}", addr_space="Shared") for g in range(2)]
            ag2_in = [[dram.tile([D_V, T // 2], B16, name=f"ag2i{j}_{hf}", tag=f"ag2i{j}_{hf}")
                       for hf in range(2)] for j in range(HL)]
            ag2_out = [[dram.tile([NCORES * D_V, T // 2], B16, name=f"ag2o{j}_{hf}",
                                  tag=f"ag2o{j}_{hf}", addr_space="Shared")
                        for hf in range(2)] for j in range(HL)]

            # ============================ Stage A
            # Ships RAW q_a / kv_a chunks (bf16) as soon as they exit PSUM; the
            # rms scales travel as bf16 hi/lo row pairs, applied consumer-side.
            with (
                tc.tile_pool(name="a_ht", bufs=1) as a_ht,
                tc.tile_pool(name="a_stage", bufs=6) as a_stage,
                tc.tile_pool(name="a_small", bufs=1) as a_small,
            ):
                ht = []
                for k in range(NKH):
                    t_ = a_ht.tile([128, TSH], B16, name=f"ht{k}", tag=f"ht{k}")
                    ht.append(t_)

                with (
                    tc.tile_pool(name="a_w", bufs=8) as a_w,
                    tc.tile_pool(name="a_sq", bufs=3) as a_sq,
                    tc.tile_pool(name="a_ps", bufs=1, space="PSUM") as a_ps,
                    tc.tile_pool(name="a_stps", bufs=1, space="PSUM") as a_stps,
                ):
                    from concourse import bass_isa

                    # ---- kv path first (feeds AG_kv early)
                    with tc.tile_pool(name="a_pspe", bufs=1, space="PSUM") as a_pspe:
                        psk = [a_ps.tile([128, TSH], F, name=f"psk{m}", tag=f"psk{m}") for m in range(NKV)]
                        pspe = a_pspe.tile([D_ROPE, TSH], F, name="pspe", tag="pspe")
                        for k in range(NKH):
                            eng = nc.sync if k % 2 == 0 else nc.scalar
                            eng.dma_start(out=ht[k][:], in_=hT[k * 128:(k + 1) * 128, :])
                            wband = a_w.tile([128, KVB], B16, name="wbandkv", tag="wbandkv")
                            eng2 = nc.scalar if k % 2 == 0 else nc.sync
                            eng2.dma_start(out=wband[:], in_=wkva[k * 128:(k + 1) * 128, :])
                            for m in range(NKV):
                                nc.tensor.matmul(
                                    psk[m][:], wband[:, m * 128:(m + 1) * 128], ht[k][:],
                                    start=(k == 0), stop=(k == NKH - 1))
                            nc.tensor.matmul(
                                pspe[:], wband[:, KV_LORA:KVB], ht[k][:],
                                start=(k == 0), stop=(k == NKH - 1))
                        kv_run = a_small.tile([128, TSH], R, name="kv_run", tag="kv_run")
                        for m in range(NKV):
                            st = a_stage.tile([128, TSH], B16, name="kvst", tag="kvst")
                            nc.vector.tensor_copy(st[:], psk[m][:])
                            nc.scalar.dma_start(out=ag_kv_in[m * 128:(m + 1) * 128, :], in_=st[:])
                            sq = a_sq.tile([128, TSH], F, name="sq2", tag="sq2")
                            nc.scalar.activation(out=sq[:], in_=psk[m][:], func=AF.Square)
                            if m == 0:
                                nc.vector.tensor_copy(kv_run[:], sq[:])
                            else:
                                nc.vector.tensor_add(kv_run[:], kv_run[:], sq[:])
                        kv_tot = a_stps.tile([1, TSH], F, name="kv_tot", tag="stat_tot")
                        nc.tensor.matmul(kv_tot[:], ones_cf[:],
                                         kv_run[:], start=True, stop=True)

                        # rope k_pe (feature-major, grouped even/odd rows)
                        cca_sb = a_small.tile([D_ROPE // 2, TSH], F, name="cca", tag="cca")
                        ssa_sb = a_small.tile([D_ROPE // 2, TSH], F, name="ssa", tag="ssa")
                        nc.scalar.dma_start(out=cca_sb[:], in_=cca[:])
                        nc.scalar.dma_start(out=ssa_sb[:], in_=ssa[:])
                        HR = D_ROPE // 2
                        kpe_sb = a_small.tile([D_ROPE, TSH], B16, name="kpe", tag="kpe")
                        t1 = a_small.tile([HR, TSH], F, name="t1", tag="t1")
                        t2 = a_small.tile([HR, TSH], F, name="t2", tag="t2")
                        nc.vector.tensor_mul(t1[:], pspe[0:HR, :], cca_sb[:])
                        nc.vector.tensor_mul(t2[:], pspe[HR:D_ROPE, :], ssa_sb[:])
                        nc.vector.tensor_sub(kpe_sb[0:HR, :], t1[:], t2[:])
                        t3 = a_small.tile([HR, TSH], F, name="t3", tag="t3")
                        t4 = a_small.tile([HR, TSH], F, name="t4", tag="t4")
                        nc.vector.tensor_mul(t3[:], pspe[HR:D_ROPE, :], cca_sb[:])
                        nc.vector.tensor_mul(t4[:], pspe[0:HR, :], ssa_sb[:])
                        nc.vector.tensor_add(kpe_sb[HR:D_ROPE, :], t3[:], t4[:])
                        nc.scalar.dma_start(out=ag_kv_in[KV_LORA:KVB, :], in_=kpe_sb[:])

                        skvr = a_small.tile([1, TSH], F, name="skvr", tag="skvr")
                        nc.scalar.activation(out=skvr[:], in_=kv_tot[:], func=AF.Sqrt,
                                             bias=eps_sb[:], scale=1.0 / KV_LORA)
                        rkv = a_small.tile([1, TSH], F, name="rkv", tag="rkv")
                        nc.vector.reciprocal(out=rkv[:], in_=skvr[:])
                        rkv_h = a_small.tile([1, TSH], B16, name="rkvh", tag="rkvh")
                        rkv_hf = a_small.tile([1, TSH], F, name="rkvhf", tag="rkvhf")
                        rkv_l = a_small.tile([1, TSH], B16, name="rkvl", tag="rkvl")
                        nc.vector.tensor_copy(rkv_h[:], rkv[:])
                        nc.vector.tensor_copy(rkv_hf[:], rkv_h[:])
                        nc.vector.tensor_sub(rkv_l[:], rkv[:], rkv_hf[:])
                        nc.scalar.dma_start(out=ag_kv_in[KVB:KVB + 1, :], in_=rkv_h[:])
                        nc.scalar.dma_start(out=ag_kv_in[KVB + 1:KVB2, :], in_=rkv_l[:])

                    nc.gpsimd.collective_compute(
                        "AllGather", mybir.AluOpType.bypass, replica_groups=RG,
                        ins=[ag_kv_in.opt()], outs=[ag_kv_out.opt()])

                    # ---- q_a path: 4 M-groups of 3 chunks reusing the kv PSUM
                    # slots; raw chunks shipped immediately; first half gathers
                    # before the stats are done.
                    qa_run = a_small.tile([128, TSH], R, name="qa_run", tag="qa_run")
                    for mg in range(2):
                        psq = ([a_ps.tile([128, TSH], F, name=f"psk{m}", tag=f"psk{m}") for m in range(4)]
                               + [a_ps.tile([128, TSH], F, name=f"psq{m}", tag=f"psq{m}") for m in range(4, 6)])
                        for k in range(NKH):
                            wband = a_w.tile([128, 768], B16, name="wband", tag="wband")
                            eng = nc.sync if k % 2 == 0 else nc.scalar
                            eng.dma_start(
                                out=wband[:],
                                in_=wqa[k * 128:(k + 1) * 128, mg * 768:(mg + 1) * 768])
                            for m in range(6):
                                nc.tensor.matmul(
                                    psq[m][:], wband[:, m * 128:(m + 1) * 128], ht[k][:],
                                    start=(k == 0), stop=(k == NKH - 1))
                        for m in range(6):
                            gm = mg * 6 + m
                            st = a_stage.tile([128, TSH], B16, name="qst", tag="qst")
                            nc.vector.tensor_copy(st[:], psq[m][:])
                            nc.scalar.dma_start(
                                out=ag_qa_in[mg][m * 128:(m + 1) * 128, :], in_=st[:])
                            sq = a_sq.tile([128, TSH], F, name="sq", tag="sq")
                            nc.scalar.activation(out=sq[:], in_=psq[m][:], func=AF.Square)
                            if gm == 0:
                                nc.vector.tensor_copy(qa_run[:], sq[:])
                            else:
                                nc.vector.tensor_add(qa_run[:], qa_run[:], sq[:])
                        if mg == 0:
                            nc.gpsimd.collective_compute(
                                "AllGather", mybir.AluOpType.bypass, replica_groups=RG,
                                ins=[ag_qa_in[0].opt()], outs=[ag_qa_out[0].opt()])
                    qa_tot = a_stps.tile([1, TSH], F, name="qa_tot", tag="stat_tot")
                    nc.tensor.matmul(qa_tot[:], ones_cf[:],
                                     qa_run[:], start=True, stop=True)
                    sqr = a_small.tile([1, TSH], F, name="sqr", tag="sqr")
                    nc.scalar.activation(out=sqr[:], in_=qa_tot[:], func=AF.Sqrt,
                                         bias=eps_sb[:], scale=1.0 / QA)
                    rq = a_small.tile([1, TSH], F, name="rq", tag="rq")
                    nc.vector.reciprocal(out=rq[:], in_=sqr[:])
                    rq_h = a_small.tile([1, TSH], B16, name="rqh", tag="rqh")
                    rq_hf = a_small.tile([1, TSH], F, name="rqhf", tag="rqhf")
                    rq_l = a_small.tile([1, TSH], B16, name="rql", tag="rql")
                    nc.vector.tensor_copy(rq_h[:], rq[:])
                    nc.vector.tensor_copy(rq_hf[:], rq_h[:])
                    nc.vector.tensor_sub(rq_l[:], rq[:], rq_hf[:])
                    nc.scalar.dma_start(out=ag_qa_in[1][QAH:QAH + 1, :], in_=rq_h[:])
                    nc.scalar.dma_start(out=ag_qa_in[1][QAH + 1:QAH2, :], in_=rq_l[:])
                    nc.gpsimd.collective_compute(
                        "AllGather", mybir.AluOpType.bypass, replica_groups=RG,
                        ins=[ag_qa_in[1].opt()], outs=[ag_qa_out[1].opt()])

            # ============================ Stage B3: k_nope / v (overlaps AG_qa)
            with tc.tile_pool(name="b_kv", bufs=1) as b_kv:
                kn = [b_kv.tile([128, T], B16, name=f"kn{h}", tag=f"kn{h}") for h in range(HL)]
                vt = [b_kv.tile([128, TB], B16, name=f"vt{i}", tag=f"vt{i}") for i in range(16)]
                # k_pe duplicated on both partition halves so both q_pe offsets
                # (0 for even heads, 64 for odd heads) have a matching lhsT
                kpe_f = b_kv.tile([128, T], B16, name="kpef", tag="kpef")
                for r in range(NCORES):
                    for half in range(2):
                        nc.sync.dma_start(
                            out=kpe_f[64 * half:64 * half + 64, r * TSH:(r + 1) * TSH],
                            in_=ag_kv_out[r * KVB2 + KV_LORA:r * KVB2 + KVB, :])

                with (
                    tc.tile_pool(name="b3_w", bufs=1) as b3_w,
                    tc.tile_pool(name="b3_s", bufs=3) as b3_s,
                    tc.tile_pool(name="b3_ps", bufs=1, space="PSUM") as b3_ps,
                    tc.tile_pool(name="b3_bc", bufs=2, space="PSUM") as b3_bc,
                ):
                    wkb = []
                    for k in range(NKV):
                        t_ = b3_w.tile([128, KB_N], B16, name=f"wkb{k}", tag=f"wkb{k}")
                        nc.sync.dma_start(out=t_[:], in_=wkvb[k * 128:(k + 1) * 128, :])
                        wkb.append(t_)

                    for nb in range(NB):
                        skvh = b3_s.tile([1, TB], B16, name="skvh", tag="skvh")
                        skvl = b3_s.tile([1, TB], B16, name="skvl", tag="skvl")
                        for half in range(2):
                            r = 2 * nb + half
                            hs = slice(half * TSH, (half + 1) * TSH)
                            nc.sync.dma_start(
                                out=skvh[:, hs], in_=ag_kv_out[r * KVB2 + KVB:r * KVB2 + KVB + 1, :])
                            nc.sync.dma_start(
                                out=skvl[:, hs], in_=ag_kv_out[r * KVB2 + KVB + 1:r * KVB2 + KVB2, :])
                        skv = b3_s.tile([1, TB], R, name="skv", tag="skv")
                        with nc.allow_low_precision(reason="fp32r rms scale for PE broadcast"):
                            nc.vector.tensor_add(skv[:], skvh[:], skvl[:])
                        skvb = b3_bc.tile([128, TB], F, name="skvb", tag="skvb")
                        nc.tensor.matmul(skvb[:], ones_rf[:],
                                         skv[:], start=True, stop=True)
                        kva = []
                        for k in range(NKV):
                            t_ = b3_s.tile([128, TB], B16, name=f"kva{k}", tag=f"kva{k}")
                            for half in range(2):
                                r = 2 * nb + half
                                eng = nc.sync if half == 0 else nc.scalar
                                eng.dma_start(
                                    out=t_[:, half * TSH:(half + 1) * TSH],
                                    in_=ag_kv_out[r * KVB2 + k * 128:r * KVB2 + (k + 1) * 128, :])
                            nc.vector.tensor_mul(t_[:], t_[:], skvb[:])
                            kva.append(t_)
                        psk = [b3_ps.tile([128, TB], F, name=f"b3k{m}", tag=f"b3k{m}") for m in range(HL)]
                        for k in range(NKV):
                            for m in range(HL):
                                nc.tensor.matmul(
                                    psk[m][:], wkb[k][:, m * 128:(m + 1) * 128], kva[k][:],
                                    start=(k == 0), stop=(k == NKV - 1))
                        for m in range(HL):
                            if m % 2 == 0:
                                nc.vector.tensor_copy(kn[m][:, nb * TB:(nb + 1) * TB], psk[m][:])
                            else:
                                nc.scalar.copy(out=kn[m][:, nb * TB:(nb + 1) * TB], in_=psk[m][:])
                        for tch in range(4):
                            psv = b3_ps.tile([128, TB], F, name="b3v", tag=f"b3k{tch}")
                            for k in range(NKV):
                                nc.tensor.matmul(
                                    psv[:], kva[k][:, tch * 128:(tch + 1) * 128],
                                    wkb[k][:, HL * D_NOPE:KB_N],
                                    start=(k == 0), stop=(k == NKV - 1))
                            nc.vector.tensor_copy(vt[nb * 4 + tch][:], psv[:])

                # ============================ Stage B1/B2: q projection + rope
                with tc.tile_pool(name="b_q", bufs=1) as b_q:
                    qn = [b_q.tile([128, T], B16, name=f"qn{h}", tag=f"qn{h}") for h in range(HL)]
                    qpe_fin = [b_q.tile([128, T], B16, name=f"qpf{t_}", tag=f"qpf{t_}") for t_ in range(2)]

                    with (
                        tc.tile_pool(name="b1_w", bufs=1) as b1_w,
                        tc.tile_pool(name="b1_s", bufs=4) as b1_s,
                        tc.tile_pool(name="b2_t", bufs=2) as b2_t,
                        tc.tile_pool(name="b1_ps", bufs=1, space="PSUM") as b1_ps,
                        tc.tile_pool(name="b2_ps", bufs=2, space="PSUM") as b2_ps,
                    ):
                        wq = []
                        for k in range(NKQ):
                            t_ = b1_w.tile([128, QB_N], B16, name=f"wq{k}", tag=f"wq{k}")
                            nc.sync.dma_start(out=t_[:], in_=wqb[k * 128:(k + 1) * 128, :])
                            wq.append(t_)
                        psw_sb = b1_w.tile([128, 128], B16, name="psw", tag="psw")
                        nc.sync.dma_start(out=psw_sb[:], in_=psw[:])

                        for nb in range(NB):
                            cs = slice(nb * TB, (nb + 1) * TB)
                            sqh = b1_s.tile([1, TB], B16, name="sqh", tag="sqh")
                            sql = b1_s.tile([1, TB], B16, name="sql", tag="sql")
                            for half in range(2):
                                r = 2 * nb + half
                                hs = slice(half * TSH, (half + 1) * TSH)
                                nc.sync.dma_start(
                                    out=sqh[:, hs],
                                    in_=ag_qa_out[1][r * QAH2 + QAH:r * QAH2 + QAH + 1, :])
                                nc.sync.dma_start(
                                    out=sql[:, hs],
                                    in_=ag_qa_out[1][r * QAH2 + QAH + 1:r * QAH2 + QAH2, :])
                            sqv = b1_s.tile([1, TB], F, name="sqv", tag="sqv")
                            nc.vector.tensor_add(sqv[:], sqh[:], sql[:])
                            rqb = b1_s.tile([128, TB], F, name="rqb", tag="rqb")
                            nc.gpsimd.partition_broadcast(rqb[:], sqv[:])
                            ps6 = [b1_ps.tile([128, TB], F, name=f"b1p{m}", tag=f"b1p{m}") for m in range(6)]
                            for k in range(NKQ):
                                g, kk = divmod(k, 6)
                                stride = QAH + 2 * g
                                rqa = b1_s.tile([128, TB], B16, name="rqa", tag="rqa")
                                for half in range(2):
                                    r = 2 * nb + half
                                    eng = nc.sync if half == 0 else nc.scalar
                                    eng.dma_start(
                                        out=rqa[:, half * TSH:(half + 1) * TSH],
                                        in_=ag_qa_out[g][r * stride + kk * 128:r * stride + (kk + 1) * 128, :])
                                for m in range(6):
                                    nc.tensor.matmul(
                                        ps6[m][:], wq[k][:, m * 128:(m + 1) * 128], rqa[:],
                                        start=(k == 0), stop=(k == NKQ - 1))
                            for m in range(HL):
                                nc.vector.tensor_mul(qn[m][:, cs], ps6[m][:], rqb[:])
                            # rope in place on the scaled q_pe
                            for t_ in range(2):
                                raw = ps6[4 + t_]
                                nc.vector.tensor_mul(qpe_fin[t_][:, cs], raw[:], rqb[:])
                                ps_sw = b2_ps.tile([128, TB], F, name="sw", tag="sw")
                                nc.tensor.matmul(ps_sw[:], psw_sb[:], qpe_fin[t_][:, cs],
                                                 start=True, stop=True)
                                ccs = b2_t.tile([128, TB], F, name="ccs", tag="ccs")
                                sss = b2_t.tile([128, TB], F, name="sss", tag="sss")
                                nc.scalar.dma_start(out=ccs[:], in_=ccq[:, cs])
                                nc.scalar.dma_start(out=sss[:], in_=ssq[:, cs])
                                tm1 = b2_t.tile([128, TB], F, name="tm1", tag="tm1")
                                tm2 = b2_t.tile([128, TB], F, name="tm2", tag="tm2")
                                nc.vector.tensor_mul(tm1[:], qpe_fin[t_][:, cs], ccs[:])
                                nc.vector.tensor_mul(tm2[:], ps_sw[:], sss[:])
                                nc.vector.tensor_add(qpe_fin[t_][:, cs], tm1[:], tm2[:])

                    # ============== Stage B4 + interleaved Stage C
                    with (
                        tc.tile_pool(name="b4_c", bufs=1) as b4_c,
                        tc.tile_pool(name="b4_e", bufs=4) as b4_e,
                        tc.tile_pool(name="b4_at", bufs=2) as b4_at,
                        tc.tile_pool(name="b4_sm", bufs=2) as b4_sm,
                        tc.tile_pool(name="c_acc", bufs=1) as c_acc,
                        tc.tile_pool(name="c_w", bufs=1) as c_w,
                        tc.tile_pool(name="c_s", bufs=1) as c_s,
                        tc.tile_pool(name="b4_ps", bufs=2, space="PSUM") as b4_ps,
                        tc.tile_pool(name="b4_po", bufs=2, space="PSUM") as b4_po,
                        tc.tile_pool(name="b4_dn", bufs=2, space="PSUM") as b4_dn,
                        tc.tile_pool(name="c_ps", bufs=2, space="PSUM") as c_ps,
                    ):
                        mask_sb = b4_c.tile([128, 4 * 512], B16, name="mask", tag="mask")
                        nc.sync.dma_start(out=mask_sb[:], in_=maskd[:])
                        acc = [c_acc.tile([128, TB], F, name=f"acc{i}", tag=f"acc{i}")
                               for i in range(16)]  # [nb*4 + mo]

                        L = 2  # out/den matmuls lag the score matmuls by L k-chunks

                        def _finish(pend_):
                            # Normalize + ship the previous (head, qj); emitted
                            # inside the next qj's score stream so the PE
                            # broadcast matmul never waits on the reciprocal.
                            h_p, qj_p, ps_o_p, ps_dn_p, recb1_p, wos_p = pend_
                            nc.tensor.matmul(ps_dn_p[:], ones_rf[:],
                                             recb1_p[:], start=True, stop=True)
                            recb_sb = b4_sm.tile([128, TB], F, name="recbb", tag="recbb")
                            nc.scalar.copy(out=recb_sb[:], in_=ps_dn_p[:])
                            at = b4_at.tile([128, TB], B16, name="at", tag="at")
                            nc.vector.tensor_mul(at[:], ps_o_p[:], recb_sb[:])
                            hf_p = qj_p // 2
                            nc.scalar.dma_start(
                                out=ag2_in[h_p][hf_p][:, (qj_p % 2) * TB:(qj_p % 2 + 1) * TB],
                                in_=at[:])
                            if qj_p % 2 != 1:
                                return
                            # this half of the head slot is complete: gather it
                            # and fold it into the output projection
                            nc.gpsimd.collective_compute(
                                "AllGather", mybir.AluOpType.bypass, replica_groups=RG,
                                ins=[ag2_in[h_p][hf_p].opt()], outs=[ag2_out[h_p][hf_p].opt()])
                            rats = []
                            for r in range(NCORES):
                                t_ = c_s.tile([128, 2 * TB], B16, name=f"rat{r}", tag=f"rat{r}")
                                nc.sync.dma_start(
                                    out=t_[:], in_=ag2_out[h_p][hf_p][r * 128:(r + 1) * 128, :])
                                rats.append(t_)
                            for nbq in range(2):
                                nb_ = hf_p * 2 + nbq
                                for mo in range(4):
                                    psc = c_ps.tile([128, TB], F, name="psc", tag="psc")
                                    for r in range(NCORES):
                                        nc.tensor.matmul(
                                            psc[:],
                                            wos_p[r][:, mo * 128:(mo + 1) * 128],
                                            rats[r][:, nbq * TB:(nbq + 1) * TB],
                                            start=(r == 0), stop=(r == NCORES - 1))
                                    a_ = acc[nb_ * 4 + mo]
                                    if h_p == 0:
                                        nc.vector.tensor_copy(a_[:], psc[:])
                                    elif h_p < HL - 1:
                                        nc.vector.tensor_add(a_[:], a_[:], psc[:])
                                    else:
                                        nc.vector.tensor_add(a_[:], a_[:], psc[:])
                                        nc.scalar.dma_start(
                                            out=out[mo * 128:(mo + 1) * 128,
                                                    nb_ * TB:(nb_ + 1) * TB],
                                            in_=a_[:])

                        pend = None
                        for h in range(HL):
                            off = 64 * (h % 2)
                            qpe_h = qpe_fin[h // 2][off:off + 64, :]
                            kpe_h = kpe_f[off:off + 64, :]
                            wos = []
                            for r in range(NCORES):
                                t_ = c_w.tile([128, WO_N], B16, name=f"wos{h % 2}_{r}",
                                              tag=f"wos{h % 2}_{r}")
                                nc.sync.dma_start(
                                    out=t_[:],
                                    in_=wo[(h * NCORES + r) * 128:(h * NCORES + r + 1) * 128, :])
                                wos.append(t_)
                            for qj in (2, 3, 0, 1):
                                qs = slice(qj * TB, (qj + 1) * TB)
                                nki = 4 * qj + 4
                                ps_o = b4_po.tile([128, TB], F, name="pso", tag="pso")
                                ps_dn = b4_dn.tile([128, TB], F, name="psdn", tag="psdn")
                                exq = []
                                for ki in range(nki + L):
                                    if ki < nki:
                                        ps_s = b4_ps.tile([128, TB], F, name="pss", tag="pss")
                                        nc.tensor.matmul(
                                            ps_s[:], kn[h][:, ki * 128:(ki + 1) * 128],
                                            qn[h][:, qs], start=True, stop=False)
                                        nc.tensor.matmul(
                                            ps_s[:], kpe_h[:, ki * 128:(ki + 1) * 128],
                                            qpe_h[:, qs], start=False, stop=True)
                                        ex = b4_e.tile([128, TB], B16, name="ex", tag="ex")
                                        nc.scalar.activation(out=ex[:], in_=ps_s[:], func=AF.Exp)
                                        d = ki - 4 * qj
                                        if d >= 0:
                                            nc.vector.tensor_mul(
                                                ex[:], ex[:], mask_sb[:, d * 512:(d + 1) * 512])
                                        exq.append(ex)
                                    if ki == 1 and pend is not None:
                                        _finish(pend)
                                        pend = None
                                    kd = ki - L
                                    if kd >= 0:
                                        nc.tensor.matmul(
                                            ps_o[:], vt[kd][:, h * 128:(h + 1) * 128], exq[kd][:],
                                            start=(kd == 0), stop=(kd == nki - 1))
                                        nc.tensor.matmul(
                                            ps_dn[0:1, :], ones_sb[:, 0:1], exq[kd][:],
                                            start=(kd == 0), stop=(kd == nki - 1))
                                recb1 = b4_sm.tile([1, TB], R, name="recb1", tag="recb1")
                                with nc.allow_low_precision(reason="fp32r softmax denom for PE broadcast"):
                                    nc.vector.reciprocal(out=recb1[:], in_=ps_dn[0:1, :])
                                pend = (h, qj, ps_o, ps_dn, recb1, wos)
                        _finish(pend)

    nc.compile()
    _CACHE["nc"] = nc
    return nc


# ---------------------------------------------------------------- host prep
def _prep_inputs(positions, hidden_states, Wqa, q_a_ln, Wqb, Wkva, kv_a_ln, Wkvb, Wo):
    import ml_dtypes

    positions = np.asarray(positions)
    hidden_states = np.ascontiguousarray(np.asarray(hidden_states, dtype=np.float32))
    Wqa = np.ascontiguousarray(np.asarray(Wqa, dtype=np.float32))
    q_a_ln = np.asarray(q_a_ln, dtype=np.float32)
    Wqb = np.asarray(Wqb, dtype=np.float32)
    Wkva = np.asarray(Wkva, dtype=np.float32)
    kv_a_ln = np.asarray(kv_a_ln, dtype=np.float32)
    Wkvb = np.asarray(Wkvb, dtype=np.float32)
    Wo = np.ascontiguousarray(np.asarray(Wo, dtype=np.float32))

    mscale = 0.1 * MSCALE_ALL_DIM * math.log(FACTOR) + 1.0
    scaling = (D_QK ** -0.5) * mscale * mscale

    inv_freq = _yarn_inv_freq()
    freqs = positions.astype(np.float32)[:, None] * inv_freq[None, :]  # [T, 32]
    cos = np.cos(freqs).astype(np.float32)
    sin = np.sin(freqs).astype(np.float32)

    HR = D_ROPE // 2
    perm = np.concatenate([np.arange(0, D_ROPE, 2), np.arange(1, D_ROPE, 2)])  # even|odd

    # Wqb: fold q_a_ln + scaling, permute per-core columns
    wqb_eff = (q_a_ln[:, None] * Wqb).reshape(QA, H, D_QK) * scaling
    wqb_cores = []
    for c in range(NCORES):
        hs = range(c * HL, (c + 1) * HL)
        cols = [wqb_eff[:, h_, :D_NOPE] for h_ in hs]
        cols += [wqb_eff[:, h_, D_NOPE + perm] for h_ in hs]
        wqb_cores.append(np.ascontiguousarray(
            np.concatenate(cols, axis=1).astype(ml_dtypes.bfloat16)))

    # Wkva: rope perm on the k_pe columns
    wkva_p = Wkva.copy()
    wkva_p[:, KV_LORA:] = Wkva[:, KV_LORA + perm]
    wkva_p = np.ascontiguousarray(wkva_p, dtype=np.float32)
    wkva_b = np.ascontiguousarray(wkva_p.astype(ml_dtypes.bfloat16))
    wqa_b = np.ascontiguousarray(Wqa.astype(ml_dtypes.bfloat16))

    # Wkvb: fold kv_a_ln, per-core [k_nope x4 | v x4]
    wkvb_eff = (kv_a_ln[:, None] * Wkvb).reshape(KV_LORA, H, D_NOPE + D_V)
    wkvb_cores = []
    for c in range(NCORES):
        hs = range(c * HL, (c + 1) * HL)
        cols = [wkvb_eff[:, h_, :D_NOPE] for h_ in hs]
        cols += [wkvb_eff[:, h_, D_NOPE:] for h_ in hs]
        wkvb_cores.append(np.ascontiguousarray(
            np.concatenate(cols, axis=1).astype(ml_dtypes.bfloat16)))

    # Wo rows permuted to the stage-C gather order: slot j, rank r -> head 4r+j
    row_order = []
    for j in range(HL):
        for r in range(NCORES):
            h_ = HL * r + j
            row_order.extend(range(h_ * D_V, (h_ + 1) * D_V))
    wo_p = Wo[row_order, :].astype(ml_dtypes.bfloat16)

    # rope ext tiles for q (2 heads per 128-row tile: [e,o | e,o] x 32 rows each)
    cosT = cos.T  # [32, T]
    sinT = sin.T
    ccq = np.ascontiguousarray(np.tile(cosT, (4, 1)), dtype=np.float32)      # [128, T]
    ssq = np.ascontiguousarray(np.concatenate([-sinT, sinT, -sinT, sinT], axis=0), dtype=np.float32)

    # swap permutation: within each 64-row block, rows 0:32 <-> 32:64
    pswm = np.zeros((128, 128), dtype=np.float32)
    for j in range(128):
        base = (j // 64) * 64
        off = j % 64
        k = base + (off + HR) % 64
        pswm[k, j] = 1.0
    pswm = pswm.astype(ml_dtypes.bfloat16)

    # causal masks for the 4 diagonal offsets (512-wide q blocks, 128-wide k chunks)
    pos = positions.astype(np.int64)
    maskd = np.zeros((128, 4 * 512), dtype=np.float32)
    for d in range(4):
        kpos = pos[d * 128:(d + 1) * 128]   # relative within a q block
        qpos = pos[0:512]
        maskd[:, d * 512:(d + 1) * 512] = (kpos[:, None] <= qpos[None, :]).astype(np.float32)
    maskd = maskd.astype(ml_dtypes.bfloat16)

    per_core = []
    for c in range(NCORES):
        sl = slice(c * TSH, (c + 1) * TSH)
        per_core.append({
            "hT": np.ascontiguousarray(hidden_states[sl].T.astype(ml_dtypes.bfloat16)),
            "wqa": wqa_b,
            "wkva": wkva_b,
            "wqb": wqb_cores[c],
            "wkvb": wkvb_cores[c],
            "wo": np.ascontiguousarray(wo_p[:, c * WO_N:(c + 1) * WO_N]),
            "cca": np.ascontiguousarray(cosT[:, sl]),
            "ssa": np.ascontiguousarray(sinT[:, sl]),
            "ccq": ccq,
            "ssq": ssq,
            "psw": pswm,
            "maskd": maskd,
            "onesd": np.ones((128, 128), dtype=ml_dtypes.bfloat16),
        })
    return per_core


def run(inputs, trace=False):
    """Build + run; returns (full_output [T, HID] fp32, exec_time_ns or None)."""
    _install_ntff_hook()
    from concourse.bass_utils import run_bass_kernel_spmd

    nc = _build_program()
    in_maps = _prep_inputs(**inputs)
    res = run_bass_kernel_spmd(nc, in_maps, list(range(NCORES)), trace=trace)
    out = np.empty((T, HID), dtype=np.float32)
    for c in range(NCORES):
        out[:, c * WO_N:(c + 1) * WO_N] = res.results[c]["out"].T
    return out, res.exec_time_ns


def kernel(**inputs):
    out, _ = run(inputs, trace=False)
    return out



# revision 14
# speedup vs baseline: 1.1368x; 1.1368x over previous
"""DeepseekV3 MLA attention kernel for 8 Trainium2 NeuronCores.

Sharding (tensor-parallel over heads + data-parallel over tokens):
  - Stage A (per core, its 256-token slice): latent = hidden @ Wkva first,
    rmsnorm + rope k_pe, AllGather it (AG_kv) while q_a = hidden @ Wqa still
    computes; q_a_n then AllGathered in two halves.  All feature-major.
    Weight-band DMAs fan out round-robin across the sync/scalar/vector/gpsimd
    queues so the PE never waits on a single DMA ring.
  - Stage B3 (overlaps AG_qa): k_nope / v = Wkvb_c.T @ kv_a_n for this core's
    4 heads, v produced token-major directly.
  - Stage B1/B2: q = Wqb_c.T @ q_a_n, rope q_pe in place (swap via PE
    permutation matmul).
  - Stage B4: causal attention per head, scores kept transposed (k on
    partitions).  Score chunks are processed in groups of two (one [128,1024]
    fp32 PSUM region spanning 2 banks) so a single Exp activation covers
    1024 columns; the out/denominator matmuls for group g are emitted inside
    group g+1's score stream (software pipelining) so the PE never waits on
    the activation.  The softmax denominator row is inverted with the fast
    approximate reciprocal, broadcast to 128 partitions with a ones-row
    matmul, and folded into the output copy.  After each head completes, its
    [128, T] attention output AllGathers (bf16) while later heads compute.
  - Stage C (separate dense phase): out_c = sum_j sum_r Wo_{j,r}.T @ rats_{j,r}
    with Wo in bf16; 8-matmul PSUM chains per (slot, out-tile), slots
    accumulated in fp32 SBUF.  Gathers for slots 0-2 complete during B4/C.

All wire payloads (AllGathers) and B-stage weights/activations are bf16; the
rms scales travel as bf16 hi/lo row pairs to keep full fp32 accuracy.  PSUM
accumulation stays fp32 throughout.
"""

import math
import sys
import types

import numpy as np

# ---------------------------------------------------------------- constants
H = 32
D_NOPE = 128
D_ROPE = 64
D_QK = 192
D_V = 128
KV_LORA = 512
EPS = 1e-6
ROPE_THETA = 10000.0
FACTOR = 40.0
BETA_FAST, BETA_SLOW = 32.0, 1.0
ORIG_MAX_POS = 4096
MSCALE_ALL_DIM = 1.0

T = 2048
HID = 4096
QA = 1536  # q lora rank
NCORES = 8
HL = H // NCORES          # 4 heads per core
TSH = T // NCORES         # 256 tokens per core
QB_N = HL * D_QK          # 768 q columns per core
KB_N = HL * (D_NOPE + D_V)  # 1024 kv columns per core
WO_N = HID // NCORES      # 512 output columns per core
KVB = KV_LORA + D_ROPE    # 576
QAH = QA // 2             # 768, AG_qa half

_CACHE = {}


def _yarn_inv_freq():
    dim = D_ROPE
    pos_freqs = ROPE_THETA ** (np.arange(0, dim, 2, dtype=np.float64) / dim)
    inv_extra = 1.0 / pos_freqs
    inv_inter = 1.0 / (FACTOR * pos_freqs)

    def corr_dim(n_rot):
        return dim * math.log(ORIG_MAX_POS / (n_rot * 2 * math.pi)) / (2 * math.log(ROPE_THETA))

    low = max(math.floor(corr_dim(BETA_FAST)), 0)
    high = min(math.ceil(corr_dim(BETA_SLOW)), dim - 1)
    ramp = np.clip((np.arange(dim // 2, dtype=np.float64) - low) / max(high - low, 1e-3), 0, 1)
    inv_freq_mask = 1.0 - ramp
    inv_freq = inv_inter * (1 - inv_freq_mask) + inv_extra * inv_freq_mask
    return inv_freq.astype(np.float32)


def _install_ntff_hook():
    """Shim antenv.axon_hooks so run_bass_kernel_spmd(trace=True) can profile."""
    if "antenv.axon_hooks" in sys.modules:
        return
    mod = types.ModuleType("antenv.axon_hooks")
    mod._hook = None

    def set_axon_ntff_profile_hook(h):
        mod._hook = h

    def get_axon_ntff_profile_hook():
        return mod._hook

    mod.set_axon_ntff_profile_hook = set_axon_ntff_profile_hook
    mod.get_axon_ntff_profile_hook = get_axon_ntff_profile_hook
    sys.modules["antenv.axon_hooks"] = mod
    try:
        import antenv

        antenv.axon_hooks = mod
        from trn_agent_boot.trn_boot import _ntff_profile_via_ctypes

        hook = _ntff_profile_via_ctypes("/opt/axon/libaxon_pjrt.so")
        if hook is not None:
            set_axon_ntff_profile_hook(hook)
    except Exception:
        pass


# ---------------------------------------------------------------- program
def _build_program():
    if "nc" in _CACHE:
        return _CACHE["nc"]

    import concourse.bacc as bacc
    import concourse.tile as tile
    from concourse import mybir

    R = mybir.dt.float32r
    F = mybir.dt.float32
    B16 = mybir.dt.bfloat16
    AF = mybir.ActivationFunctionType

    nc = bacc.Bacc("TRN2", target_bir_lowering=False, debug=False, num_devices=NCORES)

    # ------------- DRAM I/O (per-core values fed via in_maps)
    # wA = [Wkva | Wqa | hT_core] concatenated host-side: one 592 KB DMA per
    # 128-row hid chunk brings in everything stage A needs for that chunk.
    WAC = KVB + QA + TSH   # 2368 columns
    wA = nc.dram_tensor("wA", [HID, WAC], B16, kind="ExternalInput")
    wqb = nc.dram_tensor("wqb", [QA, QB_N], B16, kind="ExternalInput")
    wkvb = nc.dram_tensor("wkvb", [KV_LORA, KB_N], B16, kind="ExternalInput")
    wo = nc.dram_tensor("wo", [H * D_V, WO_N], B16, kind="ExternalInput")
    cca = nc.dram_tensor("cca", [D_ROPE // 2, TSH], F, kind="ExternalInput")
    ssa = nc.dram_tensor("ssa", [D_ROPE // 2, TSH], F, kind="ExternalInput")
    ccq = nc.dram_tensor("ccq", [128, T], F, kind="ExternalInput")
    ssq = nc.dram_tensor("ssq", [128, T], F, kind="ExternalInput")
    psw = nc.dram_tensor("psw", [128, 128], B16, kind="ExternalInput")
    maskd = nc.dram_tensor("maskd", [128, 4 * 512], B16, kind="ExternalInput")
    onesd = nc.dram_tensor("onesd", [128, 128], B16, kind="ExternalInput")
    out = nc.dram_tensor("out", [WO_N, T], F, kind="ExternalOutput")

    NKH = HID // 128   # 32 hid chunks
    NKQ = QA // 128    # 12 q-lora chunks
    NKV = KV_LORA // 128  # 4 kv-lora chunks
    NB = 4             # token blocks of 512
    TB = 512
    RG = [list(range(NCORES))]

    with tile.TileContext(nc) as tc:
        with (
            tc.tile_pool(name="consts", bufs=1) as consts,
            tc.tile_pool(name="dram", bufs=1, space="DRAM") as dram,
        ):
            ones_sb = consts.tile([128, 128], B16)
            nc.sync.dma_start(out=ones_sb[:], in_=onesd[:])
            ones_rf = consts.tile([1, 128], R)
            ones_cf = consts.tile([128, 1], R)
            with nc.allow_low_precision(reason="exact ones, fp32r for PE broadcasts"):
                nc.vector.tensor_copy(ones_rf[:], ones_sb[0:1, :])
                nc.vector.tensor_copy(ones_cf[:], ones_sb[:, 0:1])
            eps_sb = consts.tile([1, 1], F)
            nc.vector.memset(eps_sb[:], EPS)

            KVB2 = KVB + 2   # 578 rows per rank in ag_kv (576 + scale hi/lo)
            QAH2 = QAH + 2   # 770 rows per rank in ag_qa[1] (768 + scale hi/lo)
            ag_kv_in = dram.tile([KVB2, TSH], B16)
            ag_kv_out = dram.tile([NCORES * KVB2, TSH], B16, addr_space="Shared")
            ag_qa_in = [dram.tile([QAH + 2 * g, TSH], B16, name=f"agqi{g}", tag=f"agqi{g}")
                        for g in range(2)]
            ag_qa_out = [dram.tile([NCORES * (QAH + 2 * g), TSH], B16, name=f"agqo{g}",
                                   tag=f"agqo{g}", addr_space="Shared") for g in range(2)]
            ag2_in = [dram.tile([D_V, T], B16, name=f"ag2i{j}", tag=f"ag2i{j}")
                      for j in range(HL)]
            ag2_out = [dram.tile([NCORES * D_V, T], B16, name=f"ag2o{j}",
                                 tag=f"ag2o{j}", addr_space="Shared")
                       for j in range(HL)]

            # ============================ Stage A
            # Ships RAW q_a / kv_a chunks (bf16) as soon as they exit PSUM; the
            # rms scales travel as bf16 hi/lo row pairs, applied consumer-side.
            # All stage-A inputs arrive as one [128, 2368] DMA per hid chunk
            # (3 queues round-robin); the tiles stay resident so the two
            # q-projection passes re-read weights from SBUF, not HBM.
            with (
                tc.tile_pool(name="a_stage", bufs=6) as a_stage,
                tc.tile_pool(name="a_small", bufs=1) as a_small,
            ):
                with (
                    tc.tile_pool(name="a_w", bufs=1) as a_w,
                    tc.tile_pool(name="a_sq", bufs=3) as a_sq,
                    tc.tile_pool(name="a_ps", bufs=1, space="PSUM") as a_ps,
                    tc.tile_pool(name="a_stps", bufs=1, space="PSUM") as a_stps,
                ):
                    wband = []
                    for k in range(NKH):
                        t_ = a_w.tile([128, WAC], B16, name=f"wband{k}", tag=f"wband{k}")
                        eng = (nc.sync, nc.scalar, nc.gpsimd)[k % 3]
                        eng.dma_start(out=t_[:], in_=wA[k * 128:(k + 1) * 128, :])
                        wband.append(t_)
                    ht = [t_[:, KVB + QA:WAC] for t_ in wband]

                    # ---- kv path first (feeds AG_kv early)
                    with tc.tile_pool(name="a_pspe", bufs=1, space="PSUM") as a_pspe:
                        psk = [a_ps.tile([128, TSH], F, name=f"psk{m}", tag=f"psk{m}") for m in range(NKV)]
                        pspe = a_pspe.tile([D_ROPE, TSH], F, name="pspe", tag="pspe")
                        for k in range(NKH):
                            for m in range(NKV):
                                nc.tensor.matmul(
                                    psk[m][:], wband[k][:, m * 128:(m + 1) * 128], ht[k],
                                    start=(k == 0), stop=(k == NKH - 1))
                            nc.tensor.matmul(
                                pspe[:], wband[k][:, KV_LORA:KVB], ht[k],
                                start=(k == 0), stop=(k == NKH - 1))
                        kv_run = a_small.tile([128, TSH], R, name="kv_run", tag="kv_run")
                        for m in range(NKV):
                            st = a_stage.tile([128, TSH], B16, name="kvst", tag="kvst")
                            nc.vector.tensor_copy(st[:], psk[m][:])
                            nc.sync.dma_start(out=ag_kv_in[m * 128:(m + 1) * 128, :], in_=st[:])
                            sq = a_sq.tile([128, TSH], F, name="sq2", tag="sq2")
                            nc.scalar.activation(out=sq[:], in_=psk[m][:], func=AF.Square)
                            if m == 0:
                                nc.vector.tensor_copy(kv_run[:], sq[:])
                            else:
                                nc.vector.tensor_add(kv_run[:], kv_run[:], sq[:])
                        kv_tot = a_stps.tile([1, TSH], F, name="kv_tot", tag="stat_tot")
                        nc.tensor.matmul(kv_tot[:], ones_cf[:],
                                         kv_run[:], start=True, stop=True)

                        # rope k_pe (feature-major, grouped even/odd rows)
                        cca_sb = a_small.tile([D_ROPE // 2, TSH], F, name="cca", tag="cca")
                        ssa_sb = a_small.tile([D_ROPE // 2, TSH], F, name="ssa", tag="ssa")
                        nc.scalar.dma_start(out=cca_sb[:], in_=cca[:])
                        nc.scalar.dma_start(out=ssa_sb[:], in_=ssa[:])
                        HR = D_ROPE // 2
                        kpe_sb = a_small.tile([D_ROPE, TSH], B16, name="kpe", tag="kpe")
                        t1 = a_small.tile([HR, TSH], F, name="t1", tag="t1")
                        t2 = a_small.tile([HR, TSH], F, name="t2", tag="t2")
                        nc.vector.tensor_mul(t1[:], pspe[0:HR, :], cca_sb[:])
                        nc.vector.tensor_mul(t2[:], pspe[HR:D_ROPE, :], ssa_sb[:])
                        nc.vector.tensor_sub(kpe_sb[0:HR, :], t1[:], t2[:])
                        t3 = a_small.tile([HR, TSH], F, name="t3", tag="t3")
                        t4 = a_small.tile([HR, TSH], F, name="t4", tag="t4")
                        nc.vector.tensor_mul(t3[:], pspe[HR:D_ROPE, :], cca_sb[:])
                        nc.vector.tensor_mul(t4[:], pspe[0:HR, :], ssa_sb[:])
                        nc.vector.tensor_add(kpe_sb[HR:D_ROPE, :], t3[:], t4[:])
                        nc.sync.dma_start(out=ag_kv_in[KV_LORA:KVB, :], in_=kpe_sb[:])

                        skvr = a_small.tile([1, TSH], F, name="skvr", tag="skvr")
                        nc.scalar.activation(out=skvr[:], in_=kv_tot[:], func=AF.Sqrt,
                                             bias=eps_sb[:], scale=1.0 / KV_LORA)
                        rkv = a_small.tile([1, TSH], F, name="rkv", tag="rkv")
                        nc.vector.reciprocal(out=rkv[:], in_=skvr[:])
                        rkv_h = a_small.tile([1, TSH], B16, name="rkvh", tag="rkvh")
                        rkv_hf = a_small.tile([1, TSH], F, name="rkvhf", tag="rkvhf")
                        rkv_l = a_small.tile([1, TSH], B16, name="rkvl", tag="rkvl")
                        nc.vector.tensor_copy(rkv_h[:], rkv[:])
                        nc.vector.tensor_copy(rkv_hf[:], rkv_h[:])
                        nc.vector.tensor_sub(rkv_l[:], rkv[:], rkv_hf[:])
                        nc.sync.dma_start(out=ag_kv_in[KVB:KVB + 1, :], in_=rkv_h[:])
                        nc.sync.dma_start(out=ag_kv_in[KVB + 1:KVB2, :], in_=rkv_l[:])

                    nc.gpsimd.collective_compute(
                        "AllGather", mybir.AluOpType.bypass, replica_groups=RG,
                        ins=[ag_kv_in.opt()], outs=[ag_kv_out.opt()])

                    # ---- q_a path: 2 M-groups of 6 chunks reusing the kv PSUM
                    # slots; raw chunks shipped immediately; first half gathers
                    # before the stats are done.
                    qa_run = a_small.tile([128, TSH], R, name="qa_run", tag="qa_run")
                    for mg in range(2):
                        psq = ([a_ps.tile([128, TSH], F, name=f"psk{m}", tag=f"psk{m}") for m in range(4)]
                               + [a_ps.tile([128, TSH], F, name=f"psq{m}", tag=f"psq{m}") for m in range(4, 6)])
                        for k in range(NKH):
                            cb = KVB + mg * 768
                            for m in range(6):
                                nc.tensor.matmul(
                                    psq[m][:], wband[k][:, cb + m * 128:cb + (m + 1) * 128], ht[k],
                                    start=(k == 0), stop=(k == NKH - 1))
                        for m in range(6):
                            gm = mg * 6 + m
                            st = a_stage.tile([128, TSH], B16, name="qst", tag="qst")
                            nc.vector.tensor_copy(st[:], psq[m][:])
                            nc.sync.dma_start(
                                out=ag_qa_in[mg][m * 128:(m + 1) * 128, :], in_=st[:])
                            sq = a_sq.tile([128, TSH], F, name="sq", tag="sq")
                            nc.scalar.activation(out=sq[:], in_=psq[m][:], func=AF.Square)
                            if gm == 0:
                                nc.vector.tensor_copy(qa_run[:], sq[:])
                            else:
                                nc.vector.tensor_add(qa_run[:], qa_run[:], sq[:])
                        if mg == 0:
                            nc.gpsimd.collective_compute(
                                "AllGather", mybir.AluOpType.bypass, replica_groups=RG,
                                ins=[ag_qa_in[0].opt()], outs=[ag_qa_out[0].opt()])
                    qa_tot = a_stps.tile([1, TSH], F, name="qa_tot", tag="stat_tot")
                    nc.tensor.matmul(qa_tot[:], ones_cf[:],
                                     qa_run[:], start=True, stop=True)
                    sqr = a_small.tile([1, TSH], F, name="sqr", tag="sqr")
                    nc.scalar.activation(out=sqr[:], in_=qa_tot[:], func=AF.Sqrt,
                                         bias=eps_sb[:], scale=1.0 / QA)
                    rq = a_small.tile([1, TSH], F, name="rq", tag="rq")
                    nc.vector.reciprocal(out=rq[:], in_=sqr[:])
                    rq_h = a_small.tile([1, TSH], B16, name="rqh", tag="rqh")
                    rq_hf = a_small.tile([1, TSH], F, name="rqhf", tag="rqhf")
                    rq_l = a_small.tile([1, TSH], B16, name="rql", tag="rql")
                    nc.vector.tensor_copy(rq_h[:], rq[:])
                    nc.vector.tensor_copy(rq_hf[:], rq_h[:])
                    nc.vector.tensor_sub(rq_l[:], rq[:], rq_hf[:])
                    nc.sync.dma_start(out=ag_qa_in[1][QAH:QAH + 1, :], in_=rq_h[:])
                    nc.sync.dma_start(out=ag_qa_in[1][QAH + 1:QAH2, :], in_=rq_l[:])
                    nc.gpsimd.collective_compute(
                        "AllGather", mybir.AluOpType.bypass, replica_groups=RG,
                        ins=[ag_qa_in[1].opt()], outs=[ag_qa_out[1].opt()])

            # ============================ Stage B3: k_nope / v (overlaps AG_qa)
            with tc.tile_pool(name="b_kv", bufs=1) as b_kv:
                kn = [b_kv.tile([128, T], B16, name=f"kn{h}", tag=f"kn{h}") for h in range(HL)]
                vt = [b_kv.tile([128, TB], B16, name=f"vt{i}", tag=f"vt{i}") for i in range(16)]
                # k_pe duplicated on both partition halves so both q_pe offsets
                # (0 for even heads, 64 for odd heads) have a matching lhsT
                kpe_f = b_kv.tile([128, T], B16, name="kpef", tag="kpef")
                for r in range(NCORES):
                    for half in range(2):
                        nc.sync.dma_start(
                            out=kpe_f[64 * half:64 * half + 64, r * TSH:(r + 1) * TSH],
                            in_=ag_kv_out[r * KVB2 + KV_LORA:r * KVB2 + KVB, :])

                with (
                    tc.tile_pool(name="b3_w", bufs=1) as b3_w,
                    tc.tile_pool(name="b3_s", bufs=3) as b3_s,
                    tc.tile_pool(name="b3_sc", bufs=1) as b3_sc,
                    tc.tile_pool(name="b3_ps", bufs=1, space="PSUM") as b3_ps,
                    tc.tile_pool(name="b3_bc", bufs=2, space="PSUM") as b3_bc,
                ):
                    wkb = []
                    for k in range(NKV):
                        t_ = b3_w.tile([128, KB_N], B16, name=f"wkb{k}", tag=f"wkb{k}")
                        nc.gpsimd.dma_start(out=t_[:], in_=wkvb[k * 128:(k + 1) * 128, :])
                        wkb.append(t_)
                    # hi/lo rms-scale rows for all ranks, loaded once
                    skvh_all = b3_sc.tile([1, T], B16, name="skvh", tag="skvh")
                    skvl_all = b3_sc.tile([1, T], B16, name="skvl", tag="skvl")
                    for r in range(NCORES):
                        nc.scalar.dma_start(
                            out=skvh_all[0:1, r * TSH:(r + 1) * TSH],
                            in_=ag_kv_out[r * KVB2 + KVB:r * KVB2 + KVB + 1, :])
                        nc.sync.dma_start(
                            out=skvl_all[0:1, r * TSH:(r + 1) * TSH],
                            in_=ag_kv_out[r * KVB2 + KVB + 1:r * KVB2 + KVB2, :])

                    for nb in range(NB):
                        cs = slice(nb * TB, (nb + 1) * TB)
                        skv = b3_s.tile([1, TB], R, name="skv", tag="skv")
                        with nc.allow_low_precision(reason="fp32r rms scale for PE broadcast"):
                            nc.vector.tensor_add(skv[:], skvh_all[0:1, cs], skvl_all[0:1, cs])
                        skvb = b3_bc.tile([128, TB], F, name="skvb", tag="skvb")
                        nc.tensor.matmul(skvb[:], ones_rf[:],
                                         skv[:], start=True, stop=True)
                        kva = []
                        for k in range(NKV):
                            t_ = b3_s.tile([128, TB], B16, name=f"kva{k}", tag=f"kva{k}")
                            for half in range(2):
                                r = 2 * nb + half
                                eng = (nc.sync, nc.scalar, nc.gpsimd)[(2 * k + half) % 3]
                                eng.dma_start(
                                    out=t_[:, half * TSH:(half + 1) * TSH],
                                    in_=ag_kv_out[r * KVB2 + k * 128:r * KVB2 + (k + 1) * 128, :])
                            nc.vector.tensor_mul(t_[:], t_[:], skvb[:])
                            kva.append(t_)
                        psk = [b3_ps.tile([128, TB], F, name=f"b3k{m}", tag=f"b3k{m}") for m in range(HL)]
                        for k in range(NKV):
                            for m in range(HL):
                                nc.tensor.matmul(
                                    psk[m][:], wkb[k][:, m * 128:(m + 1) * 128], kva[k][:],
                                    start=(k == 0), stop=(k == NKV - 1))
                        for m in range(HL):
                            if m % 2 == 0:
                                nc.vector.tensor_copy(kn[m][:, nb * TB:(nb + 1) * TB], psk[m][:])
                            else:
                                nc.scalar.copy(out=kn[m][:, nb * TB:(nb + 1) * TB], in_=psk[m][:])
                        for tch in range(4):
                            psv = b3_ps.tile([128, TB], F, name="b3v", tag=f"b3k{tch}")
                            for k in range(NKV):
                                nc.tensor.matmul(
                                    psv[:], kva[k][:, tch * 128:(tch + 1) * 128],
                                    wkb[k][:, HL * D_NOPE:KB_N],
                                    start=(k == 0), stop=(k == NKV - 1))
                            nc.vector.tensor_copy(vt[nb * 4 + tch][:], psv[:])

                # ============================ Stage B1/B2: q projection + rope
                with tc.tile_pool(name="b_q", bufs=1) as b_q:
                    qn = [b_q.tile([128, T], B16, name=f"qn{h}", tag=f"qn{h}") for h in range(HL)]
                    qpe_fin = [b_q.tile([128, T], B16, name=f"qpf{t_}", tag=f"qpf{t_}") for t_ in range(2)]

                    with (
                        tc.tile_pool(name="b1_w", bufs=1) as b1_w,
                        tc.tile_pool(name="b1_s", bufs=4) as b1_s,
                        tc.tile_pool(name="b1_sc", bufs=1) as b1_sc,
                        tc.tile_pool(name="b2_t", bufs=2) as b2_t,
                        tc.tile_pool(name="b1_ps", bufs=1, space="PSUM") as b1_ps,
                        tc.tile_pool(name="b2_ps", bufs=2, space="PSUM") as b2_ps,
                    ):
                        wq = []
                        for k in range(NKQ):
                            t_ = b1_w.tile([128, QB_N], B16, name=f"wq{k}", tag=f"wq{k}")
                            eng = (nc.scalar, nc.gpsimd)[k % 2]
                            eng.dma_start(out=t_[:], in_=wqb[k * 128:(k + 1) * 128, :])
                            wq.append(t_)
                        psw_sb = b1_w.tile([128, 128], B16, name="psw", tag="psw")
                        nc.sync.dma_start(out=psw_sb[:], in_=psw[:])
                        # hi/lo rms-scale rows for all ranks, loaded once
                        srh = b1_sc.tile([1, T], B16, name="srh", tag="srh")
                        srl = b1_sc.tile([1, T], B16, name="srl", tag="srl")
                        for r in range(NCORES):
                            nc.scalar.dma_start(
                                out=srh[0:1, r * TSH:(r + 1) * TSH],
                                in_=ag_qa_out[1][r * QAH2 + QAH:r * QAH2 + QAH + 1, :])
                            nc.sync.dma_start(
                                out=srl[0:1, r * TSH:(r + 1) * TSH],
                                in_=ag_qa_out[1][r * QAH2 + QAH + 1:r * QAH2 + QAH2, :])
                        # all 4 token blocks' scale broadcasts up front (gpsimd)
                        rqbs = []
                        for nb in range(NB):
                            cs = slice(nb * TB, (nb + 1) * TB)
                            sqv = b1_sc.tile([1, TB], F, name=f"sqv{nb}", tag=f"sqv{nb}")
                            nc.vector.tensor_add(sqv[:], srh[0:1, cs], srl[0:1, cs])
                            rqb = b1_sc.tile([128, TB], F, name=f"rqb{nb}", tag=f"rqb{nb}")
                            nc.gpsimd.partition_broadcast(rqb[:], sqv[:])
                            rqbs.append(rqb)

                        for nb in range(NB):
                            cs = slice(nb * TB, (nb + 1) * TB)
                            rqb = rqbs[nb]
                            ps6 = [b1_ps.tile([128, TB], F, name=f"b1p{m}", tag=f"b1p{m}") for m in range(6)]
                            for k in range(NKQ):
                                g, kk = divmod(k, 6)
                                stride = QAH + 2 * g
                                rqa = b1_s.tile([128, TB], B16, name="rqa", tag="rqa")
                                for half in range(2):
                                    r = 2 * nb + half
                                    eng = (nc.sync, nc.scalar, nc.gpsimd)[(2 * k + half) % 3]
                                    eng.dma_start(
                                        out=rqa[:, half * TSH:(half + 1) * TSH],
                                        in_=ag_qa_out[g][r * stride + kk * 128:r * stride + (kk + 1) * 128, :])
                                for m in range(6):
                                    nc.tensor.matmul(
                                        ps6[m][:], wq[k][:, m * 128:(m + 1) * 128], rqa[:],
                                        start=(k == 0), stop=(k == NKQ - 1))
                            for m in range(HL):
                                nc.vector.tensor_mul(qn[m][:, cs], ps6[m][:], rqb[:])
                            # rope in place on the scaled q_pe; cos/sin tiles
                            # shared by both q_pe tiles of this token block
                            ccs = b2_t.tile([128, TB], F, name="ccs", tag="ccs")
                            sss = b2_t.tile([128, TB], F, name="sss", tag="sss")
                            nc.scalar.dma_start(out=ccs[:], in_=ccq[:, cs])
                            nc.scalar.dma_start(out=sss[:], in_=ssq[:, cs])
                            for t_ in range(2):
                                raw = ps6[4 + t_]
                                nc.vector.tensor_mul(qpe_fin[t_][:, cs], raw[:], rqb[:])
                                ps_sw = b2_ps.tile([128, TB], F, name="sw", tag="sw")
                                nc.tensor.matmul(ps_sw[:], psw_sb[:], qpe_fin[t_][:, cs],
                                                 start=True, stop=True)
                                tm1 = b2_t.tile([128, TB], F, name="tm1", tag="tm1")
                                tm2 = b2_t.tile([128, TB], F, name="tm2", tag="tm2")
                                nc.vector.tensor_mul(tm1[:], qpe_fin[t_][:, cs], ccs[:])
                                nc.vector.tensor_mul(tm2[:], ps_sw[:], sss[:])
                                nc.vector.tensor_add(qpe_fin[t_][:, cs], tm1[:], tm2[:])

                    # ============== Stage B4: causal attention, per head
                    with (
                        tc.tile_pool(name="b4_c", bufs=1) as b4_c,
                        tc.tile_pool(name="b4_at", bufs=1) as b4_at,
                        tc.tile_pool(name="b4_e", bufs=3) as b4_e,
                        tc.tile_pool(name="b4_sm", bufs=2) as b4_sm,
                        tc.tile_pool(name="b4_ps", bufs=2, space="PSUM") as b4_ps,
                        tc.tile_pool(name="b4_po", bufs=2, space="PSUM") as b4_po,
                        tc.tile_pool(name="b4_dn", bufs=2, space="PSUM") as b4_dn,
                    ):
                        mask_sb = b4_c.tile([128, 4 * 512], B16, name="mask", tag="mask")
                        nc.sync.dma_start(out=mask_sb[:], in_=maskd[:])
                        at = [b4_at.tile([D_V, T], B16, name=f"at{h}", tag=f"at{h}")
                              for h in range(HL)]

                        def _outden(st):
                            h_, qj_, g_, nki_, ex_, ps_o_, ps_dn_ = st
                            for ci in range(2):
                                ki = 2 * g_ + ci
                                nc.tensor.matmul(
                                    ps_o_[:], vt[ki][:, h_ * 128:(h_ + 1) * 128],
                                    ex_[:, ci * TB:(ci + 1) * TB],
                                    start=(ki == 0), stop=(ki == nki_ - 1))
                            for ci in range(2):
                                ki = 2 * g_ + ci
                                nc.tensor.matmul(
                                    ps_dn_[0:1, :], ones_sb[:, 0:1],
                                    ex_[:, ci * TB:(ci + 1) * TB],
                                    start=(ki == 0), stop=(ki == nki_ - 1))

                        def _norm(pn):
                            # invert the softmax denominator, broadcast it with
                            # a ones-row matmul, normalize + ship this (h, qj)
                            h_, qj_, ps_o_, ps_dn_ = pn
                            qs_ = slice(qj_ * TB, (qj_ + 1) * TB)
                            recb1r = b4_sm.tile([1, TB], R, name="recb1r", tag="recb1r")
                            with nc.allow_low_precision(reason="fp32r softmax denom for PE broadcast"):
                                nc.vector.reciprocal(out=recb1r[:], in_=ps_dn_[0:1, :])
                            nc.tensor.matmul(ps_dn_[:], ones_rf[:], recb1r[:],
                                             start=True, stop=True)
                            recb_sb = b4_sm.tile([128, TB], F, name="recbb", tag="recbb")
                            nc.scalar.copy(out=recb_sb[:], in_=ps_dn_[:])
                            nc.vector.tensor_mul(at[h_][:, qs_], ps_o_[:], recb_sb[:])
                            nc.sync.dma_start(out=ag2_in[h_][:, qs_], in_=at[h_][:, qs_])
                            if qj_ == NB - 1:
                                nc.gpsimd.collective_compute(
                                    "AllGather", mybir.AluOpType.bypass, replica_groups=RG,
                                    ins=[ag2_in[h_].opt()], outs=[ag2_out[h_].opt()])

                        pend_od = None   # score group awaiting out/den matmuls
                        pend_nm = None   # (h, qj) awaiting normalize
                        for h in range(HL):
                            off = 64 * (h % 2)
                            qpe_h = qpe_fin[h // 2][off:off + 64, :]
                            kpe_h = kpe_f[off:off + 64, :]
                            for qj in range(NB):
                                qs = slice(qj * TB, (qj + 1) * TB)
                                nki = 4 * qj + 4
                                ngrp = nki // 2
                                ps_o = b4_po.tile([128, TB], F, name="pso", tag="pso")
                                ps_dn = b4_dn.tile([128, TB], F, name="psdn", tag="psdn")
                                for g in range(ngrp):
                                    ps_s = b4_ps.tile([128, 2 * TB], F, name="pss", tag="pss")
                                    for ci in range(2):
                                        ki = 2 * g + ci
                                        ks = slice(ki * 128, (ki + 1) * 128)
                                        nc.tensor.matmul(
                                            ps_s[:, ci * TB:(ci + 1) * TB],
                                            kn[h][:, ks], qn[h][:, qs],
                                            start=True, stop=False)
                                        nc.tensor.matmul(
                                            ps_s[:, ci * TB:(ci + 1) * TB],
                                            kpe_h[:, ks], qpe_h[:, qs],
                                            start=False, stop=True)
                                    ex = b4_e.tile([128, 2 * TB], B16, name="ex", tag="ex")
                                    nc.scalar.activation(out=ex[:], in_=ps_s[:], func=AF.Exp)
                                    if g >= ngrp - 2:
                                        d0 = 2 * (g - (ngrp - 2))
                                        nc.vector.tensor_mul(
                                            ex[:], ex[:], mask_sb[:, d0 * TB:(d0 + 2) * TB])
                                    if pend_nm is not None:
                                        _norm(pend_nm)
                                        pend_nm = None
                                    if pend_od is not None:
                                        _outden(pend_od)
                                        if pend_od[2] == pend_od[3] // 2 - 1:
                                            pend_nm = (pend_od[0], pend_od[1],
                                                       pend_od[5], pend_od[6])
                                        pend_od = None
                                    pend_od = (h, qj, g, nki, ex, ps_o, ps_dn)
                        # flush the software pipeline
                        _outden(pend_od)
                        if pend_nm is not None:
                            _norm(pend_nm)
                        _norm((pend_od[0], pend_od[1], pend_od[5], pend_od[6]))

            # ============================ Stage C: output projection
            with (
                tc.tile_pool(name="c_w", bufs=2) as c_w,
                tc.tile_pool(name="c_r", bufs=2) as c_r,
                tc.tile_pool(name="c_acc", bufs=1) as c_acc,
                tc.tile_pool(name="c_ps", bufs=4, space="PSUM") as c_ps,
            ):
                acc = [c_acc.tile([128, TB], F, name=f"acc{i}", tag=f"acc{i}")
                       for i in range(16)]
                for j in range(HL):
                    wos = []
                    for r in range(NCORES):
                        t_ = c_w.tile([128, WO_N], B16, name=f"wos{r}", tag=f"wos{r}")
                        eng = nc.scalar
                        eng.dma_start(
                            out=t_[:],
                            in_=wo[(j * NCORES + r) * 128:(j * NCORES + r + 1) * 128, :])
                        wos.append(t_)
                    rats = []
                    for r in range(NCORES):
                        t_ = c_r.tile([128, T], B16, name=f"rat{r}", tag=f"rat{r}")
                        eng = (nc.sync, nc.gpsimd)[r % 2]
                        eng.dma_start(
                            out=t_[:], in_=ag2_out[j][r * 128:(r + 1) * 128, :])
                        rats.append(t_)
                    for mo in range(4):
                        for nb in range(NB):
                            psc = c_ps.tile([128, TB], F, name="psc", tag="psc")
                            for r in range(NCORES):
                                nc.tensor.matmul(
                                    psc[:],
                                    wos[r][:, mo * 128:(mo + 1) * 128],
                                    rats[r][:, nb * TB:(nb + 1) * TB],
                                    start=(r == 0), stop=(r == NCORES - 1))
                            a_ = acc[mo * 4 + nb]
                            if j == 0:
                                nc.scalar.copy(out=a_[:], in_=psc[:])
                            elif j < HL - 1:
                                nc.vector.tensor_add(a_[:], a_[:], psc[:])
                            else:
                                nc.vector.tensor_add(a_[:], a_[:], psc[:])
                                eng = (nc.sync, nc.gpsimd)[(mo * 4 + nb) % 2]
                                eng.dma_start(
                                    out=out[mo * 128:(mo + 1) * 128,
                                            nb * TB:(nb + 1) * TB],
                                    in_=a_[:])

    nc.compile()
    _CACHE["nc"] = nc
    return nc


# ---------------------------------------------------------------- host prep
def _prep_inputs(positions, hidden_states, Wqa, q_a_ln, Wqb, Wkva, kv_a_ln, Wkvb, Wo):
    import ml_dtypes

    positions = np.asarray(positions)
    hidden_states = np.ascontiguousarray(np.asarray(hidden_states, dtype=np.float32))
    Wqa = np.ascontiguousarray(np.asarray(Wqa, dtype=np.float32))
    q_a_ln = np.asarray(q_a_ln, dtype=np.float32)
    Wqb = np.asarray(Wqb, dtype=np.float32)
    Wkva = np.asarray(Wkva, dtype=np.float32)
    kv_a_ln = np.asarray(kv_a_ln, dtype=np.float32)
    Wkvb = np.asarray(Wkvb, dtype=np.float32)
    Wo = np.ascontiguousarray(np.asarray(Wo, dtype=np.float32))

    mscale = 0.1 * MSCALE_ALL_DIM * math.log(FACTOR) + 1.0
    scaling = (D_QK ** -0.5) * mscale * mscale

    inv_freq = _yarn_inv_freq()
    freqs = positions.astype(np.float32)[:, None] * inv_freq[None, :]  # [T, 32]
    cos = np.cos(freqs).astype(np.float32)
    sin = np.sin(freqs).astype(np.float32)

    HR = D_ROPE // 2
    perm = np.concatenate([np.arange(0, D_ROPE, 2), np.arange(1, D_ROPE, 2)])  # even|odd

    # Wqb: fold q_a_ln + scaling, permute per-core columns
    wqb_eff = (q_a_ln[:, None] * Wqb).reshape(QA, H, D_QK) * scaling
    wqb_cores = []
    for c in range(NCORES):
        hs = range(c * HL, (c + 1) * HL)
        cols = [wqb_eff[:, h_, :D_NOPE] for h_ in hs]
        cols += [wqb_eff[:, h_, D_NOPE + perm] for h_ in hs]
        wqb_cores.append(np.ascontiguousarray(
            np.concatenate(cols, axis=1).astype(ml_dtypes.bfloat16)))

    # Wkva: rope perm on the k_pe columns
    wkva_p = Wkva.copy()
    wkva_p[:, KV_LORA:] = Wkva[:, KV_LORA + perm]
    wkva_p = np.ascontiguousarray(wkva_p, dtype=np.float32)
    wkva_b = np.ascontiguousarray(wkva_p.astype(ml_dtypes.bfloat16))
    wqa_b = np.ascontiguousarray(Wqa.astype(ml_dtypes.bfloat16))

    # Wkvb: fold kv_a_ln, per-core [k_nope x4 | v x4]
    wkvb_eff = (kv_a_ln[:, None] * Wkvb).reshape(KV_LORA, H, D_NOPE + D_V)
    wkvb_cores = []
    for c in range(NCORES):
        hs = range(c * HL, (c + 1) * HL)
        cols = [wkvb_eff[:, h_, :D_NOPE] for h_ in hs]
        cols += [wkvb_eff[:, h_, D_NOPE:] for h_ in hs]
        wkvb_cores.append(np.ascontiguousarray(
            np.concatenate(cols, axis=1).astype(ml_dtypes.bfloat16)))

    # Wo rows permuted to the stage-C gather order: slot j, rank r -> head 4r+j
    row_order = []
    for j in range(HL):
        for r in range(NCORES):
            h_ = HL * r + j
            row_order.extend(range(h_ * D_V, (h_ + 1) * D_V))
    wo_p = Wo[row_order, :].astype(ml_dtypes.bfloat16)

    # rope ext tiles for q (2 heads per 128-row tile: [e,o | e,o] x 32 rows each)
    cosT = cos.T  # [32, T]
    sinT = sin.T
    ccq = np.ascontiguousarray(np.tile(cosT, (4, 1)), dtype=np.float32)      # [128, T]
    ssq = np.ascontiguousarray(np.concatenate([-sinT, sinT, -sinT, sinT], axis=0), dtype=np.float32)

    # swap permutation: within each 64-row block, rows 0:32 <-> 32:64
    pswm = np.zeros((128, 128), dtype=np.float32)
    for j in range(128):
        base = (j // 64) * 64
        off = j % 64
        k = base + (off + HR) % 64
        pswm[k, j] = 1.0
    pswm = pswm.astype(ml_dtypes.bfloat16)

    # causal masks for the 4 diagonal offsets (512-wide q blocks, 128-wide k chunks)
    pos = positions.astype(np.int64)
    maskd = np.zeros((128, 4 * 512), dtype=np.float32)
    for d in range(4):
        kpos = pos[d * 128:(d + 1) * 128]   # relative within a q block
        qpos = pos[0:512]
        maskd[:, d * 512:(d + 1) * 512] = (kpos[:, None] <= qpos[None, :]).astype(np.float32)
    maskd = maskd.astype(ml_dtypes.bfloat16)

    per_core = []
    for c in range(NCORES):
        sl = slice(c * TSH, (c + 1) * TSH)
        hT_c = hidden_states[sl].T.astype(ml_dtypes.bfloat16)   # [HID, TSH]
        per_core.append({
            "wA": np.ascontiguousarray(
                np.concatenate([wkva_b, wqa_b, hT_c], axis=1)),
            "wqb": wqb_cores[c],
            "wkvb": wkvb_cores[c],
            "wo": np.ascontiguousarray(wo_p[:, c * WO_N:(c + 1) * WO_N]),
            "cca": np.ascontiguousarray(cosT[:, sl]),
            "ssa": np.ascontiguousarray(sinT[:, sl]),
            "ccq": ccq,
            "ssq": ssq,
            "psw": pswm,
            "maskd": maskd,
            "onesd": np.ones((128, 128), dtype=ml_dtypes.bfloat16),
        })
    return per_core


def run(inputs, trace=False):
    """Build + run; returns (full_output [T, HID] fp32, exec_time_ns or None)."""
    _install_ntff_hook()
    from concourse.bass_utils import run_bass_kernel_spmd

    nc = _build_program()
    in_maps = _prep_inputs(**inputs)
    res = run_bass_kernel_spmd(nc, in_maps, list(range(NCORES)), trace=trace)
    out = np.empty((T, HID), dtype=np.float32)
    for c in range(NCORES):
        out[:, c * WO_N:(c + 1) * WO_N] = res.results[c]["out"].T
    return out, res.exec_time_ns


def kernel(**inputs):
    out, _ = run(inputs, trace=False)
    return out


# revision 24
# speedup vs baseline: 1.3229x; 1.1638x over previous
"""DeepseekV3 MLA attention kernel for 8 Trainium2 NeuronCores.

Sharding (tensor-parallel over heads + data-parallel over tokens):
  - Stage A (per core, its 256-token slice): latent = hidden @ Wkva first,
    rmsnorm + rope k_pe, AllGather it (AG_kv) while q_a = hidden @ Wqa still
    computes; q_a_n then AllGathered in two halves.  All feature-major.
    Weight-band DMAs fan out round-robin across the sync/scalar/vector/gpsimd
    queues so the PE never waits on a single DMA ring.
  - Stage B3 (overlaps AG_qa): k_nope / v = Wkvb_c.T @ kv_a_n for this core's
    4 heads, v produced token-major directly.
  - Stage B1/B2: q = Wqb_c.T @ q_a_n, rope q_pe in place (swap via PE
    permutation matmul).
  - Stage B4: causal attention per head, scores kept transposed (k on
    partitions).  Score chunks are processed in groups of two (one [128,1024]
    fp32 PSUM region spanning 2 banks) so a single Exp activation covers
    1024 columns; the out/denominator matmuls for group g are emitted inside
    group g+1's score stream (software pipelining) so the PE never waits on
    the activation.  The softmax denominator row is inverted with the fast
    approximate reciprocal, broadcast to 128 partitions with a ones-row
    matmul, and folded into the output copy.  After each head completes, its
    [128, T] attention output AllGathers (bf16) while later heads compute.
  - Stage C (separate dense phase): out_c = sum_j sum_r Wo_{j,r}.T @ rats_{j,r}
    with Wo in bf16; 8-matmul PSUM chains per (slot, out-tile), slots
    accumulated in fp32 SBUF.  Gathers for slots 0-2 complete during B4/C.

All wire payloads (AllGathers) and B-stage weights/activations are bf16; the
rms scales travel as bf16 hi/lo row pairs to keep full fp32 accuracy.  PSUM
accumulation stays fp32 throughout.
"""

import math
import sys
import types

import numpy as np

# ---------------------------------------------------------------- constants
H = 32
D_NOPE = 128
D_ROPE = 64
D_QK = 192
D_V = 128
KV_LORA = 512
EPS = 1e-6
ROPE_THETA = 10000.0
FACTOR = 40.0
BETA_FAST, BETA_SLOW = 32.0, 1.0
ORIG_MAX_POS = 4096
MSCALE_ALL_DIM = 1.0

T = 2048
HID = 4096
QA = 1536  # q lora rank
NCORES = 8
HL = H // NCORES          # 4 heads per core
TSH = T // NCORES         # 256 tokens per core
QB_N = HL * D_QK          # 768 q columns per core
KB_N = HL * (D_NOPE + D_V)  # 1024 kv columns per core
WO_N = HID // NCORES      # 512 output columns per core
KVB = KV_LORA + D_ROPE    # 576
QAH = QA // 2             # 768, AG_qa half

_CACHE = {}


def _yarn_inv_freq():
    dim = D_ROPE
    pos_freqs = ROPE_THETA ** (np.arange(0, dim, 2, dtype=np.float64) / dim)
    inv_extra = 1.0 / pos_freqs
    inv_inter = 1.0 / (FACTOR * pos_freqs)

    def corr_dim(n_rot):
        return dim * math.log(ORIG_MAX_POS / (n_rot * 2 * math.pi)) / (2 * math.log(ROPE_THETA))

    low = max(math.floor(corr_dim(BETA_FAST)), 0)
    high = min(math.ceil(corr_dim(BETA_SLOW)), dim - 1)
    ramp = np.clip((np.arange(dim // 2, dtype=np.float64) - low) / max(high - low, 1e-3), 0, 1)
    inv_freq_mask = 1.0 - ramp
    inv_freq = inv_inter * (1 - inv_freq_mask) + inv_extra * inv_freq_mask
    return inv_freq.astype(np.float32)


def _install_ntff_hook():
    """Shim antenv.axon_hooks so run_bass_kernel_spmd(trace=True) can profile."""
    if "antenv.axon_hooks" in sys.modules:
        return
    mod = types.ModuleType("antenv.axon_hooks")
    mod._hook = None

    def set_axon_ntff_profile_hook(h):
        mod._hook = h

    def get_axon_ntff_profile_hook():
        return mod._hook

    mod.set_axon_ntff_profile_hook = set_axon_ntff_profile_hook
    mod.get_axon_ntff_profile_hook = get_axon_ntff_profile_hook
    sys.modules["antenv.axon_hooks"] = mod
    try:
        import antenv

        antenv.axon_hooks = mod
        from trn_agent_boot.trn_boot import _ntff_profile_via_ctypes

        hook = _ntff_profile_via_ctypes("/opt/axon/libaxon_pjrt.so")
        if hook is not None:
            set_axon_ntff_profile_hook(hook)
    except Exception:
        pass


# ---------------------------------------------------------------- program
def _build_program():
    if "nc" in _CACHE:
        return _CACHE["nc"]

    import concourse.bacc as bacc
    import concourse.tile as tile
    from concourse import mybir

    R = mybir.dt.float32r
    F = mybir.dt.float32
    B16 = mybir.dt.bfloat16
    AF = mybir.ActivationFunctionType

    nc = bacc.Bacc("TRN2", target_bir_lowering=False, debug=False, num_devices=NCORES)

    # ------------- DRAM I/O (per-core values fed via in_maps)
    # wA = [Wkva | hT_core | Wqa] concatenated host-side.  Loaded in two
    # column passes: the kv-path slice [0:832] first (so AG_kv fires early),
    # then the wqa slice, prefetched while the kv matmuls run.
    WAC = KVB + QA + TSH   # 2368 columns
    wA = nc.dram_tensor("wA", [HID, WAC], B16, kind="ExternalInput")
    wqb = nc.dram_tensor("wqb", [QA, QB_N], B16, kind="ExternalInput")
    wkvb = nc.dram_tensor("wkvb", [KV_LORA, KB_N], B16, kind="ExternalInput")
    wo = nc.dram_tensor("wo", [H * D_V, WO_N], B16, kind="ExternalInput")
    cca = nc.dram_tensor("cca", [D_ROPE // 2, TSH], F, kind="ExternalInput")
    ssa = nc.dram_tensor("ssa", [D_ROPE // 2, TSH], F, kind="ExternalInput")
    ccq = nc.dram_tensor("ccq", [128, T], F, kind="ExternalInput")
    ssq = nc.dram_tensor("ssq", [128, T], F, kind="ExternalInput")
    psw = nc.dram_tensor("psw", [128, 128], B16, kind="ExternalInput")
    maskd = nc.dram_tensor("maskd", [128, 4 * 512], B16, kind="ExternalInput")
    onesd = nc.dram_tensor("onesd", [128, 128], B16, kind="ExternalInput")
    out = nc.dram_tensor("out", [WO_N, T], F, kind="ExternalOutput")

    NKH = HID // 128   # 32 hid chunks
    NKQ = QA // 128    # 12 q-lora chunks
    NKV = KV_LORA // 128  # 4 kv-lora chunks
    NB = 4             # token blocks of 512
    TB = 512
    RG = [list(range(NCORES))]

    with tile.TileContext(nc) as tc:
        with (
            tc.tile_pool(name="consts", bufs=1) as consts,
            tc.tile_pool(name="dram", bufs=1, space="DRAM") as dram,
        ):
            ones_sb = consts.tile([128, 128], B16)
            nc.sync.dma_start(out=ones_sb[:], in_=onesd[:])
            ones_rf = consts.tile([1, 128], R)
            ones_cf = consts.tile([128, 1], R)
            with nc.allow_low_precision(reason="exact ones, fp32r for PE broadcasts"):
                nc.vector.tensor_copy(ones_rf[:], ones_sb[0:1, :])
                nc.vector.tensor_copy(ones_cf[:], ones_sb[:, 0:1])
            eps_sb = consts.tile([1, 1], F)
            nc.vector.memset(eps_sb[:], EPS)

            KVB2 = KVB + 2   # 578 rows per rank in ag_kv (576 + scale hi/lo)
            QAH2 = QAH + 2   # 770 rows per rank in ag_qa[1] (768 + scale hi/lo)
            ag_kv_in = dram.tile([KVB2, TSH], B16)
            ag_kv_out = dram.tile([NCORES * KVB2, TSH], B16, addr_space="Shared")
            ag_qa_in = [dram.tile([QAH + 2 * g, TSH], B16, name=f"agqi{g}", tag=f"agqi{g}")
                        for g in range(2)]
            ag_qa_out = [dram.tile([NCORES * (QAH + 2 * g), TSH], B16, name=f"agqo{g}",
                                   tag=f"agqo{g}", addr_space="Shared") for g in range(2)]
            ag2_in = [dram.tile([D_V, T], B16, name=f"ag2i{j}", tag=f"ag2i{j}")
                      for j in range(HL)]
            ag2_out = [dram.tile([NCORES * D_V, T], B16, name=f"ag2o{j}",
                                 tag=f"ag2o{j}", addr_space="Shared")
                       for j in range(HL)]

            # ============================ Stage A
            # Ships RAW q_a / kv_a chunks (bf16) as soon as they exit PSUM; the
            # rms scales travel as bf16 hi/lo row pairs, applied consumer-side.
            # All stage-A inputs arrive as one [128, 2368] DMA per hid chunk
            # (3 queues round-robin); the tiles stay resident so the two
            # q-projection passes re-read weights from SBUF, not HBM.
            with (
                tc.tile_pool(name="a_stage", bufs=6) as a_stage,
                tc.tile_pool(name="a_small", bufs=1) as a_small,
            ):
                with (
                    tc.tile_pool(name="a_w", bufs=1) as a_w,
                    tc.tile_pool(name="a_sq", bufs=3) as a_sq,
                    tc.tile_pool(name="a_ps", bufs=1, space="PSUM") as a_ps,
                    tc.tile_pool(name="a_stps", bufs=1, space="PSUM") as a_stps,
                ):
                    HTB = KVB + TSH   # 832: end of the kv+hT column block
                    wband = []
                    for k in range(NKH):
                        t_ = a_w.tile([128, WAC], B16, name=f"wband{k}", tag=f"wband{k}")
                        eng = (nc.sync, nc.scalar, nc.gpsimd)[k % 3]
                        eng.dma_start(out=t_[:, 0:HTB], in_=wA[k * 128:(k + 1) * 128, 0:HTB])
                        wband.append(t_)
                    for k in range(NKH):
                        eng = (nc.sync, nc.scalar, nc.gpsimd)[k % 3]
                        eng.dma_start(out=wband[k][:, HTB:WAC],
                                      in_=wA[k * 128:(k + 1) * 128, HTB:WAC])
                    ht = [t_[:, KVB:HTB] for t_ in wband]

                    # ---- kv path first (feeds AG_kv early)
                    with tc.tile_pool(name="a_pspe", bufs=1, space="PSUM") as a_pspe:
                        psk = [a_ps.tile([128, TSH], F, name=f"psk{m}", tag=f"psk{m}") for m in range(NKV)]
                        pspe = a_pspe.tile([D_ROPE, TSH], F, name="pspe", tag="pspe")
                        for k in range(NKH):
                            for m in range(NKV):
                                nc.tensor.matmul(
                                    psk[m][:], wband[k][:, m * 128:(m + 1) * 128], ht[k],
                                    start=(k == 0), stop=(k == NKH - 1))
                            nc.tensor.matmul(
                                pspe[:], wband[k][:, KV_LORA:KVB], ht[k],
                                start=(k == 0), stop=(k == NKH - 1))
                        kv_run = a_small.tile([128, TSH], R, name="kv_run", tag="kv_run")
                        for m in range(NKV):
                            st = a_stage.tile([128, TSH], B16, name="kvst", tag="kvst")
                            nc.vector.tensor_copy(st[:], psk[m][:])
                            nc.sync.dma_start(out=ag_kv_in[m * 128:(m + 1) * 128, :], in_=st[:])
                            sq = a_sq.tile([128, TSH], F, name="sq2", tag="sq2")
                            nc.scalar.activation(out=sq[:], in_=psk[m][:], func=AF.Square)
                            if m == 0:
                                nc.vector.tensor_copy(kv_run[:], sq[:])
                            else:
                                nc.vector.tensor_add(kv_run[:], kv_run[:], sq[:])
                        kv_tot = a_stps.tile([1, TSH], F, name="kv_tot", tag="stat_tot")
                        nc.tensor.matmul(kv_tot[:], ones_cf[:],
                                         kv_run[:], start=True, stop=True)

                        # rope k_pe (feature-major, grouped even/odd rows)
                        cca_sb = a_small.tile([D_ROPE // 2, TSH], F, name="cca", tag="cca")
                        ssa_sb = a_small.tile([D_ROPE // 2, TSH], F, name="ssa", tag="ssa")
                        nc.scalar.dma_start(out=cca_sb[:], in_=cca[:])
                        nc.scalar.dma_start(out=ssa_sb[:], in_=ssa[:])
                        HR = D_ROPE // 2
                        kpe_sb = a_small.tile([D_ROPE, TSH], B16, name="kpe", tag="kpe")
                        t1 = a_small.tile([HR, TSH], F, name="t1", tag="t1")
                        t2 = a_small.tile([HR, TSH], F, name="t2", tag="t2")
                        nc.vector.tensor_mul(t1[:], pspe[0:HR, :], cca_sb[:])
                        nc.vector.tensor_mul(t2[:], pspe[HR:D_ROPE, :], ssa_sb[:])
                        nc.vector.tensor_sub(kpe_sb[0:HR, :], t1[:], t2[:])
                        t3 = a_small.tile([HR, TSH], F, name="t3", tag="t3")
                        t4 = a_small.tile([HR, TSH], F, name="t4", tag="t4")
                        nc.vector.tensor_mul(t3[:], pspe[HR:D_ROPE, :], cca_sb[:])
                        nc.vector.tensor_mul(t4[:], pspe[0:HR, :], ssa_sb[:])
                        nc.vector.tensor_add(kpe_sb[HR:D_ROPE, :], t3[:], t4[:])
                        nc.sync.dma_start(out=ag_kv_in[KV_LORA:KVB, :], in_=kpe_sb[:])

                        skvr = a_small.tile([1, TSH], F, name="skvr", tag="skvr")
                        nc.scalar.activation(out=skvr[:], in_=kv_tot[:], func=AF.Sqrt,
                                             bias=eps_sb[:], scale=1.0 / KV_LORA)
                        rkv = a_small.tile([1, TSH], F, name="rkv", tag="rkv")
                        nc.vector.reciprocal(out=rkv[:], in_=skvr[:])
                        rkv_h = a_small.tile([1, TSH], B16, name="rkvh", tag="rkvh")
                        rkv_hf = a_small.tile([1, TSH], F, name="rkvhf", tag="rkvhf")
                        rkv_l = a_small.tile([1, TSH], B16, name="rkvl", tag="rkvl")
                        nc.vector.tensor_copy(rkv_h[:], rkv[:])
                        nc.vector.tensor_copy(rkv_hf[:], rkv_h[:])
                        nc.vector.tensor_sub(rkv_l[:], rkv[:], rkv_hf[:])
                        nc.sync.dma_start(out=ag_kv_in[KVB:KVB + 1, :], in_=rkv_h[:])
                        nc.sync.dma_start(out=ag_kv_in[KVB + 1:KVB2, :], in_=rkv_l[:])

                    nc.gpsimd.collective_compute(
                        "AllGather", mybir.AluOpType.bypass, replica_groups=RG,
                        ins=[ag_kv_in.opt()], outs=[ag_kv_out.opt()])

                    # ---- q_a path: 2 M-groups of 6 chunks reusing the kv PSUM
                    # slots; raw chunks shipped immediately; first half gathers
                    # before the stats are done.
                    qa_run = a_small.tile([128, TSH], R, name="qa_run", tag="qa_run")
                    for mg in range(2):
                        psq = ([a_ps.tile([128, TSH], F, name=f"psk{m}", tag=f"psk{m}") for m in range(4)]
                               + [a_ps.tile([128, TSH], F, name=f"psq{m}", tag=f"psq{m}") for m in range(4, 6)])
                        for k in range(NKH):
                            cb = HTB + mg * 768
                            for m in range(6):
                                nc.tensor.matmul(
                                    psq[m][:], wband[k][:, cb + m * 128:cb + (m + 1) * 128], ht[k],
                                    start=(k == 0), stop=(k == NKH - 1))
                        for m in range(6):
                            gm = mg * 6 + m
                            st = a_stage.tile([128, TSH], B16, name="qst", tag="qst")
                            nc.vector.tensor_copy(st[:], psq[m][:])
                            nc.sync.dma_start(
                                out=ag_qa_in[mg][m * 128:(m + 1) * 128, :], in_=st[:])
                            sq = a_sq.tile([128, TSH], F, name="sq", tag="sq")
                            nc.scalar.activation(out=sq[:], in_=psq[m][:], func=AF.Square)
                            if gm == 0:
                                nc.vector.tensor_copy(qa_run[:], sq[:])
                            else:
                                nc.vector.tensor_add(qa_run[:], qa_run[:], sq[:])
                        if mg == 0:
                            nc.gpsimd.collective_compute(
                                "AllGather", mybir.AluOpType.bypass, replica_groups=RG,
                                ins=[ag_qa_in[0].opt()], outs=[ag_qa_out[0].opt()])
                    qa_tot = a_stps.tile([1, TSH], F, name="qa_tot", tag="stat_tot")
                    nc.tensor.matmul(qa_tot[:], ones_cf[:],
                                     qa_run[:], start=True, stop=True)
                    sqr = a_small.tile([1, TSH], F, name="sqr", tag="sqr")
                    nc.scalar.activation(out=sqr[:], in_=qa_tot[:], func=AF.Sqrt,
                                         bias=eps_sb[:], scale=1.0 / QA)
                    rq = a_small.tile([1, TSH], F, name="rq", tag="rq")
                    nc.vector.reciprocal(out=rq[:], in_=sqr[:])
                    rq_h = a_small.tile([1, TSH], B16, name="rqh", tag="rqh")
                    rq_hf = a_small.tile([1, TSH], F, name="rqhf", tag="rqhf")
                    rq_l = a_small.tile([1, TSH], B16, name="rql", tag="rql")
                    nc.vector.tensor_copy(rq_h[:], rq[:])
                    nc.vector.tensor_copy(rq_hf[:], rq_h[:])
                    nc.vector.tensor_sub(rq_l[:], rq[:], rq_hf[:])
                    nc.sync.dma_start(out=ag_qa_in[1][QAH:QAH + 1, :], in_=rq_h[:])
                    nc.sync.dma_start(out=ag_qa_in[1][QAH + 1:QAH2, :], in_=rq_l[:])
                    nc.gpsimd.collective_compute(
                        "AllGather", mybir.AluOpType.bypass, replica_groups=RG,
                        ins=[ag_qa_in[1].opt()], outs=[ag_qa_out[1].opt()])

            # ============================ Stage B3: k_nope / v (overlaps AG_qa)
            with tc.tile_pool(name="b_kv", bufs=1) as b_kv:
                kn = [b_kv.tile([128, T], B16, name=f"kn{h}", tag=f"kn{h}") for h in range(HL)]
                vt = [b_kv.tile([128, TB], B16, name=f"vt{i}", tag=f"vt{i}") for i in range(16)]
                # k_pe duplicated on both partition halves so both q_pe offsets
                # (0 for even heads, 64 for odd heads) have a matching lhsT
                kpe_f = b_kv.tile([128, T], B16, name="kpef", tag="kpef")
                for r in range(NCORES):
                    for half in range(2):
                        nc.sync.dma_start(
                            out=kpe_f[64 * half:64 * half + 64, r * TSH:(r + 1) * TSH],
                            in_=ag_kv_out[r * KVB2 + KV_LORA:r * KVB2 + KVB, :])

                with (
                    tc.tile_pool(name="b3_w", bufs=1) as b3_w,
                    tc.tile_pool(name="b3_s", bufs=3) as b3_s,
                    tc.tile_pool(name="b3_sc", bufs=1) as b3_sc,
                    tc.tile_pool(name="b3_ps", bufs=1, space="PSUM") as b3_ps,
                    tc.tile_pool(name="b3_bc", bufs=2, space="PSUM") as b3_bc,
                ):
                    wkb = []
                    for k in range(NKV):
                        t_ = b3_w.tile([128, KB_N], B16, name=f"wkb{k}", tag=f"wkb{k}")
                        nc.gpsimd.dma_start(out=t_[:], in_=wkvb[k * 128:(k + 1) * 128, :])
                        wkb.append(t_)
                    # hi/lo rms-scale rows for all ranks, loaded once
                    skvh_all = b3_sc.tile([1, T], B16, name="skvh", tag="skvh")
                    skvl_all = b3_sc.tile([1, T], B16, name="skvl", tag="skvl")
                    for r in range(NCORES):
                        nc.scalar.dma_start(
                            out=skvh_all[0:1, r * TSH:(r + 1) * TSH],
                            in_=ag_kv_out[r * KVB2 + KVB:r * KVB2 + KVB + 1, :])
                        nc.sync.dma_start(
                            out=skvl_all[0:1, r * TSH:(r + 1) * TSH],
                            in_=ag_kv_out[r * KVB2 + KVB + 1:r * KVB2 + KVB2, :])

                    for nb in range(NB):
                        cs = slice(nb * TB, (nb + 1) * TB)
                        skv = b3_s.tile([1, TB], R, name="skv", tag="skv")
                        with nc.allow_low_precision(reason="fp32r rms scale for PE broadcast"):
                            nc.vector.tensor_add(skv[:], skvh_all[0:1, cs], skvl_all[0:1, cs])
                        skvb = b3_bc.tile([128, TB], F, name="skvb", tag="skvb")
                        nc.tensor.matmul(skvb[:], ones_rf[:],
                                         skv[:], start=True, stop=True)
                        kva = []
                        for k in range(NKV):
                            t_ = b3_s.tile([128, TB], B16, name=f"kva{k}", tag=f"kva{k}")
                            for half in range(2):
                                r = 2 * nb + half
                                eng = (nc.sync, nc.scalar, nc.gpsimd)[(2 * k + half) % 3]
                                eng.dma_start(
                                    out=t_[:, half * TSH:(half + 1) * TSH],
                                    in_=ag_kv_out[r * KVB2 + k * 128:r * KVB2 + (k + 1) * 128, :])
                            nc.vector.tensor_mul(t_[:], t_[:], skvb[:])
                            kva.append(t_)
                        psk = [b3_ps.tile([128, TB], F, name=f"b3k{m}", tag=f"b3k{m}") for m in range(HL)]
                        for k in range(NKV):
                            for m in range(HL):
                                nc.tensor.matmul(
                                    psk[m][:], wkb[k][:, m * 128:(m + 1) * 128], kva[k][:],
                                    start=(k == 0), stop=(k == NKV - 1))
                        for m in range(HL):
                            if m % 2 == 0:
                                nc.vector.tensor_copy(kn[m][:, nb * TB:(nb + 1) * TB], psk[m][:])
                            else:
                                nc.scalar.copy(out=kn[m][:, nb * TB:(nb + 1) * TB], in_=psk[m][:])
                        for tch in range(4):
                            psv = b3_ps.tile([128, TB], F, name="b3v", tag=f"b3k{tch}")
                            for k in range(NKV):
                                nc.tensor.matmul(
                                    psv[:], kva[k][:, tch * 128:(tch + 1) * 128],
                                    wkb[k][:, HL * D_NOPE:KB_N],
                                    start=(k == 0), stop=(k == NKV - 1))
                            nc.vector.tensor_copy(vt[nb * 4 + tch][:], psv[:])

                # ============================ Stage B1/B2: q projection + rope
                with tc.tile_pool(name="b_q", bufs=1) as b_q:
                    qn = [b_q.tile([128, T], B16, name=f"qn{h}", tag=f"qn{h}") for h in range(HL)]
                    qpe_fin = [b_q.tile([128, T], B16, name=f"qpf{t_}", tag=f"qpf{t_}") for t_ in range(2)]

                    with (
                        tc.tile_pool(name="b1_w", bufs=1) as b1_w,
                        tc.tile_pool(name="b1_s", bufs=4) as b1_s,
                        tc.tile_pool(name="b1_sc", bufs=1) as b1_sc,
                        tc.tile_pool(name="b2_t", bufs=2) as b2_t,
                        tc.tile_pool(name="b1_ps", bufs=1, space="PSUM") as b1_ps,
                        tc.tile_pool(name="b2_ps", bufs=2, space="PSUM") as b2_ps,
                    ):
                        wq = []
                        for k in range(NKQ):
                            t_ = b1_w.tile([128, QB_N], B16, name=f"wq{k}", tag=f"wq{k}")
                            eng = (nc.scalar, nc.gpsimd)[k % 2]
                            eng.dma_start(out=t_[:], in_=wqb[k * 128:(k + 1) * 128, :])
                            wq.append(t_)
                        psw_sb = b1_w.tile([128, 128], B16, name="psw", tag="psw")
                        nc.sync.dma_start(out=psw_sb[:], in_=psw[:])
                        # hi/lo rms-scale rows for all ranks, loaded once
                        srh = b1_sc.tile([1, T], B16, name="srh", tag="srh")
                        srl = b1_sc.tile([1, T], B16, name="srl", tag="srl")
                        for r in range(NCORES):
                            nc.scalar.dma_start(
                                out=srh[0:1, r * TSH:(r + 1) * TSH],
                                in_=ag_qa_out[1][r * QAH2 + QAH:r * QAH2 + QAH + 1, :])
                            nc.sync.dma_start(
                                out=srl[0:1, r * TSH:(r + 1) * TSH],
                                in_=ag_qa_out[1][r * QAH2 + QAH + 1:r * QAH2 + QAH2, :])
                        # all 4 token blocks' scale broadcasts up front (gpsimd)
                        rqbs = []
                        for nb in range(NB):
                            cs = slice(nb * TB, (nb + 1) * TB)
                            sqv = b1_sc.tile([1, TB], F, name=f"sqv{nb}", tag=f"sqv{nb}")
                            nc.vector.tensor_add(sqv[:], srh[0:1, cs], srl[0:1, cs])
                            rqb = b1_sc.tile([128, TB], F, name=f"rqb{nb}", tag=f"rqb{nb}")
                            nc.gpsimd.partition_broadcast(rqb[:], sqv[:])
                            rqbs.append(rqb)

                        for nb in range(NB):
                            cs = slice(nb * TB, (nb + 1) * TB)
                            rqb = rqbs[nb]
                            ps6 = [b1_ps.tile([128, TB], F, name=f"b1p{m}", tag=f"b1p{m}") for m in range(6)]
                            for k in range(NKQ):
                                g, kk = divmod(k, 6)
                                stride = QAH + 2 * g
                                rqa = b1_s.tile([128, TB], B16, name="rqa", tag="rqa")
                                for half in range(2):
                                    r = 2 * nb + half
                                    eng = (nc.sync, nc.scalar, nc.gpsimd)[(2 * k + half) % 3]
                                    eng.dma_start(
                                        out=rqa[:, half * TSH:(half + 1) * TSH],
                                        in_=ag_qa_out[g][r * stride + kk * 128:r * stride + (kk + 1) * 128, :])
                                for m in range(6):
                                    nc.tensor.matmul(
                                        ps6[m][:], wq[k][:, m * 128:(m + 1) * 128], rqa[:],
                                        start=(k == 0), stop=(k == NKQ - 1))
                            for m in range(HL):
                                nc.vector.tensor_mul(qn[m][:, cs], ps6[m][:], rqb[:])
                            # rope in place on the scaled q_pe; cos/sin tiles
                            # shared by both q_pe tiles of this token block
                            ccs = b2_t.tile([128, TB], F, name="ccs", tag="ccs")
                            sss = b2_t.tile([128, TB], F, name="sss", tag="sss")
                            nc.scalar.dma_start(out=ccs[:], in_=ccq[:, cs])
                            nc.scalar.dma_start(out=sss[:], in_=ssq[:, cs])
                            for t_ in range(2):
                                raw = ps6[4 + t_]
                                nc.vector.tensor_mul(qpe_fin[t_][:, cs], raw[:], rqb[:])
                                ps_sw = b2_ps.tile([128, TB], F, name="sw", tag="sw")
                                nc.tensor.matmul(ps_sw[:], psw_sb[:], qpe_fin[t_][:, cs],
                                                 start=True, stop=True)
                                tm1 = b2_t.tile([128, TB], F, name="tm1", tag="tm1")
                                tm2 = b2_t.tile([128, TB], F, name="tm2", tag="tm2")
                                nc.vector.tensor_mul(tm1[:], qpe_fin[t_][:, cs], ccs[:])
                                nc.vector.tensor_mul(tm2[:], ps_sw[:], sss[:])
                                nc.vector.tensor_add(qpe_fin[t_][:, cs], tm1[:], tm2[:])

                    # qsw: qpe_fin with its 64-row halves swapped (SBUF-to-SBUF
                    # partition-shift DMAs) so every head's q_pe exists at both
                    # partition offsets; paired kpe score matmuls then run on
                    # disjoint PE row-groups (concurrent in the array).
                    qsw = [b_q.tile([128, T], B16, name=f"qsw{t_}", tag=f"qsw{t_}")
                           for t_ in range(2)]
                    for t_ in range(2):
                        nc.sync.dma_start(out=qsw[t_][0:64, :], in_=qpe_fin[t_][64:128, :])
                        nc.scalar.dma_start(out=qsw[t_][64:128, :], in_=qpe_fin[t_][0:64, :])

                    import contextlib
                    cstack = contextlib.ExitStack()
                    c_w = cstack.enter_context(tc.tile_pool(name="c_w", bufs=2))
                    c_r = cstack.enter_context(tc.tile_pool(name="c_r", bufs=2))
                    c_pref = {}

                    # ============== Stage B4: causal attention, per head
                    with (
                        tc.tile_pool(name="b4_c", bufs=1) as b4_c,
                        tc.tile_pool(name="b4_at", bufs=1) as b4_at,
                        tc.tile_pool(name="b4_e", bufs=3) as b4_e,
                        tc.tile_pool(name="b4_sm", bufs=2) as b4_sm,
                        tc.tile_pool(name="b4_ps", bufs=2, space="PSUM") as b4_ps,
                        tc.tile_pool(name="b4_po", bufs=2, space="PSUM") as b4_po,
                        tc.tile_pool(name="b4_dn", bufs=2, space="PSUM") as b4_dn,
                    ):
                        mask_sb = b4_c.tile([128, 4 * 512], B16, name="mask", tag="mask")
                        nc.sync.dma_start(out=mask_sb[:], in_=maskd[:])
                        at = [b4_at.tile([D_V, T], B16, name=f"at{h}", tag=f"at{h}")
                              for h in range(HL)]

                        def _outden(st):
                            h_, qj_, g_, nki_, ex_, ps_o_, ps_dn_ = st
                            for ci in range(2):
                                ki = 2 * g_ + ci
                                nc.tensor.matmul(
                                    ps_o_[:], vt[ki][:, h_ * 128:(h_ + 1) * 128],
                                    ex_[:, ci * TB:(ci + 1) * TB],
                                    start=(ki == 0), stop=(ki == nki_ - 1))
                            for ci in range(2):
                                ki = 2 * g_ + ci
                                nc.tensor.matmul(
                                    ps_dn_[0:1, :], ones_sb[:, 0:1],
                                    ex_[:, ci * TB:(ci + 1) * TB],
                                    start=(ki == 0), stop=(ki == nki_ - 1))

                        def _norm(pn):
                            # invert the softmax denominator as exp(-ln(x)) on
                            # the scalar engine (both funcs share one activation
                            # table set), broadcast it with a ones-row matmul,
                            # normalize + ship this (h, qj)
                            h_, qj_, ps_o_, ps_dn_ = pn
                            qs_ = slice(qj_ * TB, (qj_ + 1) * TB)
                            lden = b4_sm.tile([1, TB], F, name="lden", tag="lden")
                            nc.scalar.activation(out=lden[:], in_=ps_dn_[0:1, :], func=AF.Ln)
                            recb1r = b4_sm.tile([1, TB], R, name="recb1r", tag="recb1r")
                            with nc.allow_low_precision(reason="fp32r softmax denom for PE broadcast"):
                                nc.scalar.activation(out=recb1r[:], in_=lden[:],
                                                     func=AF.Exp, scale=-1.0)
                            nc.tensor.matmul(ps_dn_[:], ones_rf[:], recb1r[:],
                                             start=True, stop=True)
                            recb_sb = b4_sm.tile([128, TB], F, name="recbb", tag="recbb")
                            nc.vector.tensor_copy(recb_sb[:], ps_dn_[:])
                            nc.vector.tensor_mul(at[h_][:, qs_], ps_o_[:], recb_sb[:])
                            nc.sync.dma_start(out=ag2_in[h_][:, qs_], in_=at[h_][:, qs_])
                            if qj_ == NB - 1:
                                nc.gpsimd.collective_compute(
                                    "AllGather", mybir.AluOpType.bypass, replica_groups=RG,
                                    ins=[ag2_in[h_].opt()], outs=[ag2_out[h_].opt()])
                                if h_ == 0:
                                    # prefetch stage C's slot-0 weights and
                                    # gathered activations while B4 continues
                                    pw, pr = [], []
                                    for r_ in range(NCORES):
                                        t_ = c_w.tile([128, WO_N], B16,
                                                      name=f"wos{r_}", tag=f"wos{r_}")
                                        nc.scalar.dma_start(
                                            out=t_[:], in_=wo[r_ * 128:(r_ + 1) * 128, :])
                                        pw.append(t_)
                                        t_ = c_r.tile([128, T], B16,
                                                      name=f"rat{r_}", tag=f"rat{r_}")
                                        nc.gpsimd.dma_start(
                                            out=t_[:], in_=ag2_out[0][r_ * 128:(r_ + 1) * 128, :])
                                        pr.append(t_)
                                    c_pref["wos"] = pw
                                    c_pref["rats"] = pr

                        pend_od = None   # score group awaiting out/den matmuls
                        pend_nm = None   # (h, qj) awaiting normalize
                        for h in range(HL):
                            t0 = h // 2
                            # this head's q_pe at both partition offsets
                            qpe_b0 = (qpe_fin[t0] if h % 2 == 0 else qsw[t0])
                            qpe_b64 = (qsw[t0] if h % 2 == 0 else qpe_fin[t0])
                            for qj in range(NB):
                                qs = slice(qj * TB, (qj + 1) * TB)
                                nki = 4 * qj + 4
                                ngrp = nki // 2
                                ps_o = b4_po.tile([128, TB], F, name="pso", tag="pso")
                                ps_dn = b4_dn.tile([128, TB], F, name="psdn", tag="psdn")
                                for g in range(ngrp):
                                    ps_s = b4_ps.tile([128, 2 * TB], F, name="pss", tag="pss")
                                    ks0 = slice(2 * g * 128, (2 * g + 1) * 128)
                                    ks1 = slice((2 * g + 1) * 128, (2 * g + 2) * 128)
                                    nc.tensor.matmul(
                                        ps_s[:, 0:TB], kn[h][:, ks0], qn[h][:, qs],
                                        start=True, stop=False)
                                    nc.tensor.matmul(
                                        ps_s[:, TB:2 * TB], kn[h][:, ks1], qn[h][:, qs],
                                        start=True, stop=False)
                                    # the two 64-deep rope matmuls sit on
                                    # disjoint PE row-groups -> run concurrently
                                    nc.tensor.matmul(
                                        ps_s[:, 0:TB], kpe_f[0:64, ks0],
                                        qpe_b0[0:64, qs], start=False, stop=True)
                                    nc.tensor.matmul(
                                        ps_s[:, TB:2 * TB], kpe_f[64:128, ks1],
                                        qpe_b64[64:128, qs], start=False, stop=True)
                                    ex = b4_e.tile([128, 2 * TB], B16, name="ex", tag="ex")
                                    nc.scalar.activation(out=ex[:], in_=ps_s[:], func=AF.Exp)
                                    if g >= ngrp - 2:
                                        d0 = 2 * (g - (ngrp - 2))
                                        nc.vector.tensor_mul(
                                            ex[:], ex[:], mask_sb[:, d0 * TB:(d0 + 2) * TB])
                                    if pend_nm is not None:
                                        _norm(pend_nm)
                                        pend_nm = None
                                    if pend_od is not None:
                                        _outden(pend_od)
                                        if pend_od[2] == pend_od[3] // 2 - 1:
                                            pend_nm = (pend_od[0], pend_od[1],
                                                       pend_od[5], pend_od[6])
                                        pend_od = None
                                    pend_od = (h, qj, g, nki, ex, ps_o, ps_dn)
                        # flush the software pipeline
                        _outden(pend_od)
                        if pend_nm is not None:
                            _norm(pend_nm)
                        _norm((pend_od[0], pend_od[1], pend_od[5], pend_od[6]))

                    # ============== Stage C: output projection (dense phase)
                    with (
                        tc.tile_pool(name="c_acc", bufs=1) as c_acc,
                        tc.tile_pool(name="c_ps", bufs=4, space="PSUM") as c_ps,
                    ):
                        acc = [c_acc.tile([128, TB], F, name=f"acc{i}", tag=f"acc{i}")
                               for i in range(16)]
                        for j in range(HL):
                            if j == 0:
                                wos = c_pref["wos"]
                                rats = c_pref["rats"]
                            else:
                                wos = []
                                for r in range(NCORES):
                                    t_ = c_w.tile([128, WO_N], B16, name=f"wos{r}", tag=f"wos{r}")
                                    nc.scalar.dma_start(
                                        out=t_[:],
                                        in_=wo[(j * NCORES + r) * 128:(j * NCORES + r + 1) * 128, :])
                                    wos.append(t_)
                                rats = []
                                for r in range(NCORES):
                                    t_ = c_r.tile([128, T], B16, name=f"rat{r}", tag=f"rat{r}")
                                    eng = (nc.sync, nc.gpsimd)[r % 2]
                                    eng.dma_start(
                                        out=t_[:], in_=ag2_out[j][r * 128:(r + 1) * 128, :])
                                    rats.append(t_)
                            for mo in range(4):
                                for nb in range(NB):
                                    psc = c_ps.tile([128, TB], F, name="psc", tag="psc")
                                    for r in range(NCORES):
                                        nc.tensor.matmul(
                                            psc[:],
                                            wos[r][:, mo * 128:(mo + 1) * 128],
                                            rats[r][:, nb * TB:(nb + 1) * TB],
                                            start=(r == 0), stop=(r == NCORES - 1))
                                    a_ = acc[mo * 4 + nb]
                                    if j == 0:
                                        nc.scalar.copy(out=a_[:], in_=psc[:])
                                    elif j < HL - 1:
                                        nc.vector.tensor_add(a_[:], a_[:], psc[:])
                                    else:
                                        nc.vector.tensor_add(a_[:], a_[:], psc[:])
                                        eng = (nc.sync, nc.gpsimd)[(mo * 4 + nb) % 2]
                                        eng.dma_start(
                                            out=out[mo * 128:(mo + 1) * 128,
                                                    nb * TB:(nb + 1) * TB],
                                            in_=a_[:])
                    cstack.close()

    nc.compile()
    _CACHE["nc"] = nc
    return nc


# ---------------------------------------------------------------- host prep
def _prep_inputs(positions, hidden_states, Wqa, q_a_ln, Wqb, Wkva, kv_a_ln, Wkvb, Wo):
    import ml_dtypes

    positions = np.asarray(positions)
    hidden_states = np.ascontiguousarray(np.asarray(hidden_states, dtype=np.float32))
    Wqa = np.ascontiguousarray(np.asarray(Wqa, dtype=np.float32))
    q_a_ln = np.asarray(q_a_ln, dtype=np.float32)
    Wqb = np.asarray(Wqb, dtype=np.float32)
    Wkva = np.asarray(Wkva, dtype=np.float32)
    kv_a_ln = np.asarray(kv_a_ln, dtype=np.float32)
    Wkvb = np.asarray(Wkvb, dtype=np.float32)
    Wo = np.ascontiguousarray(np.asarray(Wo, dtype=np.float32))

    mscale = 0.1 * MSCALE_ALL_DIM * math.log(FACTOR) + 1.0
    scaling = (D_QK ** -0.5) * mscale * mscale

    inv_freq = _yarn_inv_freq()
    freqs = positions.astype(np.float32)[:, None] * inv_freq[None, :]  # [T, 32]
    cos = np.cos(freqs).astype(np.float32)
    sin = np.sin(freqs).astype(np.float32)

    HR = D_ROPE // 2
    perm = np.concatenate([np.arange(0, D_ROPE, 2), np.arange(1, D_ROPE, 2)])  # even|odd

    # Wqb: fold q_a_ln + scaling, permute per-core columns
    wqb_eff = (q_a_ln[:, None] * Wqb).reshape(QA, H, D_QK) * scaling
    wqb_cores = []
    for c in range(NCORES):
        hs = range(c * HL, (c + 1) * HL)
        cols = [wqb_eff[:, h_, :D_NOPE] for h_ in hs]
        cols += [wqb_eff[:, h_, D_NOPE + perm] for h_ in hs]
        wqb_cores.append(np.ascontiguousarray(
            np.concatenate(cols, axis=1).astype(ml_dtypes.bfloat16)))

    # Wkva: rope perm on the k_pe columns
    wkva_p = Wkva.copy()
    wkva_p[:, KV_LORA:] = Wkva[:, KV_LORA + perm]
    wkva_p = np.ascontiguousarray(wkva_p, dtype=np.float32)
    wkva_b = np.ascontiguousarray(wkva_p.astype(ml_dtypes.bfloat16))
    wqa_b = np.ascontiguousarray(Wqa.astype(ml_dtypes.bfloat16))

    # Wkvb: fold kv_a_ln, per-core [k_nope x4 | v x4]
    wkvb_eff = (kv_a_ln[:, None] * Wkvb).reshape(KV_LORA, H, D_NOPE + D_V)
    wkvb_cores = []
    for c in range(NCORES):
        hs = range(c * HL, (c + 1) * HL)
        cols = [wkvb_eff[:, h_, :D_NOPE] for h_ in hs]
        cols += [wkvb_eff[:, h_, D_NOPE:] for h_ in hs]
        wkvb_cores.append(np.ascontiguousarray(
            np.concatenate(cols, axis=1).astype(ml_dtypes.bfloat16)))

    # Wo rows permuted to the stage-C gather order: slot j, rank r -> head 4r+j
    row_order = []
    for j in range(HL):
        for r in range(NCORES):
            h_ = HL * r + j
            row_order.extend(range(h_ * D_V, (h_ + 1) * D_V))
    wo_p = Wo[row_order, :].astype(ml_dtypes.bfloat16)

    # rope ext tiles for q (2 heads per 128-row tile: [e,o | e,o] x 32 rows each)
    cosT = cos.T  # [32, T]
    sinT = sin.T
    ccq = np.ascontiguousarray(np.tile(cosT, (4, 1)), dtype=np.float32)      # [128, T]
    ssq = np.ascontiguousarray(np.concatenate([-sinT, sinT, -sinT, sinT], axis=0), dtype=np.float32)

    # swap permutation: within each 64-row block, rows 0:32 <-> 32:64
    pswm = np.zeros((128, 128), dtype=np.float32)
    for j in range(128):
        base = (j // 64) * 64
        off = j % 64
        k = base + (off + HR) % 64
        pswm[k, j] = 1.0
    pswm = pswm.astype(ml_dtypes.bfloat16)

    # causal masks for the 4 diagonal offsets (512-wide q blocks, 128-wide k chunks)
    pos = positions.astype(np.int64)
    maskd = np.zeros((128, 4 * 512), dtype=np.float32)
    for d in range(4):
        kpos = pos[d * 128:(d + 1) * 128]   # relative within a q block
        qpos = pos[0:512]
        maskd[:, d * 512:(d + 1) * 512] = (kpos[:, None] <= qpos[None, :]).astype(np.float32)
    maskd = maskd.astype(ml_dtypes.bfloat16)

    per_core = []
    for c in range(NCORES):
        sl = slice(c * TSH, (c + 1) * TSH)
        hT_c = hidden_states[sl].T.astype(ml_dtypes.bfloat16)   # [HID, TSH]
        per_core.append({
            "wA": np.ascontiguousarray(
                np.concatenate([wkva_b, hT_c, wqa_b], axis=1)),
            "wqb": wqb_cores[c],
            "wkvb": wkvb_cores[c],
            "wo": np.ascontiguousarray(wo_p[:, c * WO_N:(c + 1) * WO_N]),
            "cca": np.ascontiguousarray(cosT[:, sl]),
            "ssa": np.ascontiguousarray(sinT[:, sl]),
            "ccq": ccq,
            "ssq": ssq,
            "psw": pswm,
            "maskd": maskd,
            "onesd": np.ones((128, 128), dtype=ml_dtypes.bfloat16),
        })
    return per_core


def run(inputs, trace=False):
    """Build + run; returns (full_output [T, HID] fp32, exec_time_ns or None)."""
    _install_ntff_hook()
    from concourse.bass_utils import run_bass_kernel_spmd

    nc = _build_program()
    in_maps = _prep_inputs(**inputs)
    res = run_bass_kernel_spmd(nc, in_maps, list(range(NCORES)), trace=trace)
    out = np.empty((T, HID), dtype=np.float32)
    for c in range(NCORES):
        out[:, c * WO_N:(c + 1) * WO_N] = res.results[c]["out"].T
    return out, res.exec_time_ns


def kernel(**inputs):
    out, _ = run(inputs, trace=False)
    return out


# revision 30
# speedup vs baseline: 1.3377x; 1.0112x over previous
"""DeepseekV3 MLA attention kernel for 8 Trainium2 NeuronCores.

Sharding (tensor-parallel over heads + data-parallel over tokens):
  - Stage A (per core, its 256-token slice): latent = hidden @ Wkva first,
    rmsnorm + rope k_pe, AllGather it (AG_kv) while q_a = hidden @ Wqa still
    computes; q_a_n then AllGathered in two halves.  All feature-major.
    Weight-band DMAs fan out round-robin across the sync/scalar/vector/gpsimd
    queues so the PE never waits on a single DMA ring.
  - Stage B3 (overlaps AG_qa): k_nope / v = Wkvb_c.T @ kv_a_n for this core's
    4 heads, v produced token-major directly.
  - Stage B1/B2: q = Wqb_c.T @ q_a_n, rope q_pe in place (swap via PE
    permutation matmul).
  - Stage B4: causal attention per head, scores kept transposed (k on
    partitions).  Score chunks are processed in groups of two (one [128,1024]
    fp32 PSUM region spanning 2 banks) so a single Exp activation covers
    1024 columns; the out/denominator matmuls for group g are emitted inside
    group g+1's score stream (software pipelining) so the PE never waits on
    the activation.  The softmax denominator row is inverted with the fast
    approximate reciprocal, broadcast to 128 partitions with a ones-row
    matmul, and folded into the output copy.  After each head completes, its
    [128, T] attention output AllGathers (bf16) while later heads compute.
  - Stage C (separate dense phase): out_c = sum_j sum_r Wo_{j,r}.T @ rats_{j,r}
    with Wo in bf16; 8-matmul PSUM chains per (slot, out-tile), slots
    accumulated in fp32 SBUF.  Gathers for slots 0-2 complete during B4/C.

All wire payloads (AllGathers) and B-stage weights/activations are bf16; the
rms scales travel as bf16 hi/lo row pairs to keep full fp32 accuracy.  PSUM
accumulation stays fp32 throughout.
"""

import math
import sys
import types

import numpy as np

# ---------------------------------------------------------------- constants
H = 32
D_NOPE = 128
D_ROPE = 64
D_QK = 192
D_V = 128
KV_LORA = 512
EPS = 1e-6
ROPE_THETA = 10000.0
FACTOR = 40.0
BETA_FAST, BETA_SLOW = 32.0, 1.0
ORIG_MAX_POS = 4096
MSCALE_ALL_DIM = 1.0

T = 2048
HID = 4096
QA = 1536  # q lora rank
NCORES = 8
HL = H // NCORES          # 4 heads per core
TSH = T // NCORES         # 256 tokens per core
QB_N = HL * D_QK          # 768 q columns per core
KB_N = HL * (D_NOPE + D_V)  # 1024 kv columns per core
WO_N = HID // NCORES      # 512 output columns per core
KVB = KV_LORA + D_ROPE    # 576
QAH = QA // 2             # 768, AG_qa half

_CACHE = {}


def _yarn_inv_freq():
    dim = D_ROPE
    pos_freqs = ROPE_THETA ** (np.arange(0, dim, 2, dtype=np.float64) / dim)
    inv_extra = 1.0 / pos_freqs
    inv_inter = 1.0 / (FACTOR * pos_freqs)

    def corr_dim(n_rot):
        return dim * math.log(ORIG_MAX_POS / (n_rot * 2 * math.pi)) / (2 * math.log(ROPE_THETA))

    low = max(math.floor(corr_dim(BETA_FAST)), 0)
    high = min(math.ceil(corr_dim(BETA_SLOW)), dim - 1)
    ramp = np.clip((np.arange(dim // 2, dtype=np.float64) - low) / max(high - low, 1e-3), 0, 1)
    inv_freq_mask = 1.0 - ramp
    inv_freq = inv_inter * (1 - inv_freq_mask) + inv_extra * inv_freq_mask
    return inv_freq.astype(np.float32)


def _install_ntff_hook():
    """Shim antenv.axon_hooks so run_bass_kernel_spmd(trace=True) can profile."""
    if "antenv.axon_hooks" in sys.modules:
        return
    mod = types.ModuleType("antenv.axon_hooks")
    mod._hook = None

    def set_axon_ntff_profile_hook(h):
        mod._hook = h

    def get_axon_ntff_profile_hook():
        return mod._hook

    mod.set_axon_ntff_profile_hook = set_axon_ntff_profile_hook
    mod.get_axon_ntff_profile_hook = get_axon_ntff_profile_hook
    sys.modules["antenv.axon_hooks"] = mod
    try:
        import antenv

        antenv.axon_hooks = mod
        from trn_agent_boot.trn_boot import _ntff_profile_via_ctypes

        hook = _ntff_profile_via_ctypes("/opt/axon/libaxon_pjrt.so")
        if hook is not None:
            set_axon_ntff_profile_hook(hook)
    except Exception:
        pass


# ---------------------------------------------------------------- program
def _build_program():
    if "nc" in _CACHE:
        return _CACHE["nc"]

    import concourse.bacc as bacc
    import concourse.tile as tile
    from concourse import mybir

    R = mybir.dt.float32r
    F = mybir.dt.float32
    B16 = mybir.dt.bfloat16
    AF = mybir.ActivationFunctionType

    nc = bacc.Bacc("TRN2", target_bir_lowering=False, debug=False, num_devices=NCORES)

    # ------------- DRAM I/O (per-core values fed via in_maps)
    # wA = [Wkva | hT_core | Wqa] concatenated host-side.  Loaded in two
    # column passes: the kv-path slice [0:832] first (so AG_kv fires early),
    # then the wqa slice, prefetched while the kv matmuls run.
    WAC = KVB + QA + TSH   # 2368 columns
    wA = nc.dram_tensor("wA", [HID, WAC], B16, kind="ExternalInput")
    wqb = nc.dram_tensor("wqb", [QA, QB_N], B16, kind="ExternalInput")
    wkvb = nc.dram_tensor("wkvb", [KV_LORA, KB_N], B16, kind="ExternalInput")
    wo = nc.dram_tensor("wo", [H * D_V, WO_N], B16, kind="ExternalInput")
    cca = nc.dram_tensor("cca", [D_ROPE // 2, TSH], F, kind="ExternalInput")
    ssa = nc.dram_tensor("ssa", [D_ROPE // 2, TSH], F, kind="ExternalInput")
    ccq = nc.dram_tensor("ccq", [128, T], F, kind="ExternalInput")
    ssq = nc.dram_tensor("ssq", [128, T], F, kind="ExternalInput")
    psw = nc.dram_tensor("psw", [128, 128], B16, kind="ExternalInput")
    maskd = nc.dram_tensor("maskd", [128, 4 * 512], B16, kind="ExternalInput")
    onesd = nc.dram_tensor("onesd", [128, 128], B16, kind="ExternalInput")
    out = nc.dram_tensor("out", [WO_N, T], F, kind="ExternalOutput")

    NKH = HID // 128   # 32 hid chunks
    NKQ = QA // 128    # 12 q-lora chunks
    NKV = KV_LORA // 128  # 4 kv-lora chunks
    NB = 4             # token blocks of 512
    TB = 512
    RG = [list(range(NCORES))]

    with tile.TileContext(nc) as tc:
        with (
            tc.tile_pool(name="consts", bufs=1) as consts,
            tc.tile_pool(name="dram", bufs=1, space="DRAM") as dram,
        ):
            ones_sb = consts.tile([128, 128], B16)
            nc.sync.dma_start(out=ones_sb[:], in_=onesd[:])
            ones_rf = consts.tile([1, 128], R)
            ones_cf = consts.tile([128, 1], R)
            with nc.allow_low_precision(reason="exact ones, fp32r for PE broadcasts"):
                nc.vector.tensor_copy(ones_rf[:], ones_sb[0:1, :])
                nc.vector.tensor_copy(ones_cf[:], ones_sb[:, 0:1])
            eps_sb = consts.tile([1, 1], F)
            nc.vector.memset(eps_sb[:], EPS)

            KVB2 = KVB + 2   # 578 rows per rank in ag_kv (576 + scale hi/lo)
            QAH2 = QAH + 2   # 770 rows per rank in ag_qa[1] (768 + scale hi/lo)
            ag_kv_in = dram.tile([KVB2, TSH], B16)
            ag_kv_out = dram.tile([NCORES * KVB2, TSH], B16, addr_space="Shared")
            ag_qa_in = [dram.tile([QAH + 2 * g, TSH], B16, name=f"agqi{g}", tag=f"agqi{g}")
                        for g in range(2)]
            ag_qa_out = [dram.tile([NCORES * (QAH + 2 * g), TSH], B16, name=f"agqo{g}",
                                   tag=f"agqo{g}", addr_space="Shared") for g in range(2)]
            ag2_in = [dram.tile([D_V, T], B16, name=f"ag2i{j}", tag=f"ag2i{j}")
                      for j in range(HL)]
            ag2_out = [dram.tile([NCORES * D_V, T], B16, name=f"ag2o{j}",
                                 tag=f"ag2o{j}", addr_space="Shared")
                       for j in range(HL)]

            # ============================ Stage A
            # Ships RAW q_a / kv_a chunks (bf16) as soon as they exit PSUM; the
            # rms scales travel as bf16 hi/lo row pairs, applied consumer-side.
            # All stage-A inputs arrive as one [128, 2368] DMA per hid chunk
            # (3 queues round-robin); the tiles stay resident so the two
            # q-projection passes re-read weights from SBUF, not HBM.
            with (
                tc.tile_pool(name="a_stage", bufs=6) as a_stage,
                tc.tile_pool(name="a_small", bufs=1) as a_small,
            ):
                with (
                    tc.tile_pool(name="a_w", bufs=1) as a_w,
                    tc.tile_pool(name="a_sq", bufs=3) as a_sq,
                    tc.tile_pool(name="a_ps", bufs=1, space="PSUM") as a_ps,
                    tc.tile_pool(name="a_stps", bufs=1, space="PSUM") as a_stps,
                ):
                    # bulk weight loads stay off the sync queue so stage A's
                    # small critical DMAs (rope tables, staging, scales) are
                    # never stuck behind them
                    HTB = KVB + TSH   # 832: end of the kv+hT column block
                    wband = []
                    for k in range(NKH):
                        t_ = a_w.tile([128, WAC], B16, name=f"wband{k}", tag=f"wband{k}")
                        eng = (nc.scalar, nc.gpsimd)[k % 2]
                        eng.dma_start(out=t_[:, 0:HTB], in_=wA[k * 128:(k + 1) * 128, 0:HTB])
                        wband.append(t_)
                    for k in range(NKH):
                        eng = (nc.scalar, nc.gpsimd)[k % 2]
                        eng.dma_start(out=wband[k][:, HTB:WAC],
                                      in_=wA[k * 128:(k + 1) * 128, HTB:WAC])
                    ht = [t_[:, KVB:HTB] for t_ in wband]

                    # ---- kv path first (feeds AG_kv early)
                    with tc.tile_pool(name="a_pspe", bufs=1, space="PSUM") as a_pspe:
                        psk = [a_ps.tile([128, TSH], F, name=f"psk{m}", tag=f"psk{m}") for m in range(NKV)]
                        pspe = a_pspe.tile([D_ROPE, TSH], F, name="pspe", tag="pspe")
                        for k in range(NKH):
                            for m in range(NKV):
                                nc.tensor.matmul(
                                    psk[m][:], wband[k][:, m * 128:(m + 1) * 128], ht[k],
                                    start=(k == 0), stop=(k == NKH - 1))
                            nc.tensor.matmul(
                                pspe[:], wband[k][:, KV_LORA:KVB], ht[k],
                                start=(k == 0), stop=(k == NKH - 1))
                        kv_run = a_small.tile([128, TSH], R, name="kv_run", tag="kv_run")
                        for m in range(NKV):
                            st = a_stage.tile([128, TSH], B16, name="kvst", tag="kvst")
                            nc.vector.tensor_copy(st[:], psk[m][:])
                            nc.sync.dma_start(out=ag_kv_in[m * 128:(m + 1) * 128, :], in_=st[:])
                            sq = a_sq.tile([128, TSH], F, name="sq2", tag="sq2")
                            nc.scalar.activation(out=sq[:], in_=psk[m][:], func=AF.Square)
                            if m == 0:
                                nc.vector.tensor_copy(kv_run[:], sq[:])
                            else:
                                nc.vector.tensor_add(kv_run[:], kv_run[:], sq[:])
                        kv_tot = a_stps.tile([1, TSH], F, name="kv_tot", tag="stat_tot")
                        nc.tensor.matmul(kv_tot[:], ones_cf[:],
                                         kv_run[:], start=True, stop=True)

                        # rope k_pe (feature-major, grouped even/odd rows)
                        cca_sb = a_small.tile([D_ROPE // 2, TSH], F, name="cca", tag="cca")
                        ssa_sb = a_small.tile([D_ROPE // 2, TSH], F, name="ssa", tag="ssa")
                        nc.sync.dma_start(out=cca_sb[:], in_=cca[:])
                        nc.sync.dma_start(out=ssa_sb[:], in_=ssa[:])
                        HR = D_ROPE // 2
                        kpe_sb = a_small.tile([D_ROPE, TSH], B16, name="kpe", tag="kpe")
                        t1 = a_small.tile([HR, TSH], F, name="t1", tag="t1")
                        t2 = a_small.tile([HR, TSH], F, name="t2", tag="t2")
                        nc.vector.tensor_mul(t1[:], pspe[0:HR, :], cca_sb[:])
                        nc.vector.tensor_mul(t2[:], pspe[HR:D_ROPE, :], ssa_sb[:])
                        nc.vector.tensor_sub(kpe_sb[0:HR, :], t1[:], t2[:])
                        t3 = a_small.tile([HR, TSH], F, name="t3", tag="t3")
                        t4 = a_small.tile([HR, TSH], F, name="t4", tag="t4")
                        nc.vector.tensor_mul(t3[:], pspe[HR:D_ROPE, :], cca_sb[:])
                        nc.vector.tensor_mul(t4[:], pspe[0:HR, :], ssa_sb[:])
                        nc.vector.tensor_add(kpe_sb[HR:D_ROPE, :], t3[:], t4[:])
                        nc.sync.dma_start(out=ag_kv_in[KV_LORA:KVB, :], in_=kpe_sb[:])

                        skvr = a_small.tile([1, TSH], F, name="skvr", tag="skvr")
                        nc.scalar.activation(out=skvr[:], in_=kv_tot[:], func=AF.Sqrt,
                                             bias=eps_sb[:], scale=1.0 / KV_LORA)
                        rkv = a_small.tile([1, TSH], F, name="rkv", tag="rkv")
                        nc.vector.reciprocal(out=rkv[:], in_=skvr[:])
                        rkv_h = a_small.tile([1, TSH], B16, name="rkvh", tag="rkvh")
                        rkv_hf = a_small.tile([1, TSH], F, name="rkvhf", tag="rkvhf")
                        rkv_l = a_small.tile([1, TSH], B16, name="rkvl", tag="rkvl")
                        nc.vector.tensor_copy(rkv_h[:], rkv[:])
                        nc.vector.tensor_copy(rkv_hf[:], rkv_h[:])
                        nc.vector.tensor_sub(rkv_l[:], rkv[:], rkv_hf[:])
                        nc.sync.dma_start(out=ag_kv_in[KVB:KVB + 1, :], in_=rkv_h[:])
                        nc.sync.dma_start(out=ag_kv_in[KVB + 1:KVB2, :], in_=rkv_l[:])

                    nc.gpsimd.collective_compute(
                        "AllGather", mybir.AluOpType.bypass, replica_groups=RG,
                        ins=[ag_kv_in.opt()], outs=[ag_kv_out.opt()])

                    # ---- q_a path: 2 M-groups of 6 chunks reusing the kv PSUM
                    # slots; raw chunks shipped immediately; first half gathers
                    # before the stats are done.
                    qa_run = a_small.tile([128, TSH], R, name="qa_run", tag="qa_run")
                    for mg in range(2):
                        psq = ([a_ps.tile([128, TSH], F, name=f"psk{m}", tag=f"psk{m}") for m in range(4)]
                               + [a_ps.tile([128, TSH], F, name=f"psq{m}", tag=f"psq{m}") for m in range(4, 6)])
                        for k in range(NKH):
                            cb = HTB + mg * 768
                            for m in range(6):
                                nc.tensor.matmul(
                                    psq[m][:], wband[k][:, cb + m * 128:cb + (m + 1) * 128], ht[k],
                                    start=(k == 0), stop=(k == NKH - 1))
                        for m in range(6):
                            gm = mg * 6 + m
                            st = a_stage.tile([128, TSH], B16, name="qst", tag="qst")
                            nc.vector.tensor_copy(st[:], psq[m][:])
                            nc.sync.dma_start(
                                out=ag_qa_in[mg][m * 128:(m + 1) * 128, :], in_=st[:])
                            sq = a_sq.tile([128, TSH], F, name="sq", tag="sq")
                            nc.scalar.activation(out=sq[:], in_=psq[m][:], func=AF.Square)
                            if gm == 0:
                                nc.vector.tensor_copy(qa_run[:], sq[:])
                            else:
                                nc.vector.tensor_add(qa_run[:], qa_run[:], sq[:])
                        if mg == 0:
                            nc.gpsimd.collective_compute(
                                "AllGather", mybir.AluOpType.bypass, replica_groups=RG,
                                ins=[ag_qa_in[0].opt()], outs=[ag_qa_out[0].opt()])
                    qa_tot = a_stps.tile([1, TSH], F, name="qa_tot", tag="stat_tot")
                    nc.tensor.matmul(qa_tot[:], ones_cf[:],
                                     qa_run[:], start=True, stop=True)
                    sqr = a_small.tile([1, TSH], F, name="sqr", tag="sqr")
                    nc.scalar.activation(out=sqr[:], in_=qa_tot[:], func=AF.Sqrt,
                                         bias=eps_sb[:], scale=1.0 / QA)
                    rq = a_small.tile([1, TSH], F, name="rq", tag="rq")
                    nc.vector.reciprocal(out=rq[:], in_=sqr[:])
                    rq_h = a_small.tile([1, TSH], B16, name="rqh", tag="rqh")
                    rq_hf = a_small.tile([1, TSH], F, name="rqhf", tag="rqhf")
                    rq_l = a_small.tile([1, TSH], B16, name="rql", tag="rql")
                    nc.vector.tensor_copy(rq_h[:], rq[:])
                    nc.vector.tensor_copy(rq_hf[:], rq_h[:])
                    nc.vector.tensor_sub(rq_l[:], rq[:], rq_hf[:])
                    nc.sync.dma_start(out=ag_qa_in[1][QAH:QAH + 1, :], in_=rq_h[:])
                    nc.sync.dma_start(out=ag_qa_in[1][QAH + 1:QAH2, :], in_=rq_l[:])
                    nc.gpsimd.collective_compute(
                        "AllGather", mybir.AluOpType.bypass, replica_groups=RG,
                        ins=[ag_qa_in[1].opt()], outs=[ag_qa_out[1].opt()])

            # ============================ Stage B3: k_nope / v (overlaps AG_qa)
            with tc.tile_pool(name="b_kv", bufs=1) as b_kv:
                kn = [b_kv.tile([128, T], B16, name=f"kn{h}", tag=f"kn{h}") for h in range(HL)]
                vt = [b_kv.tile([128, TB], B16, name=f"vt{i}", tag=f"vt{i}") for i in range(16)]
                # k_pe duplicated on both partition halves so both q_pe offsets
                # (0 for even heads, 64 for odd heads) have a matching lhsT
                kpe_f = b_kv.tile([128, T], B16, name="kpef", tag="kpef")
                for r in range(NCORES):
                    for half in range(2):
                        nc.sync.dma_start(
                            out=kpe_f[64 * half:64 * half + 64, r * TSH:(r + 1) * TSH],
                            in_=ag_kv_out[r * KVB2 + KV_LORA:r * KVB2 + KVB, :])

                with (
                    tc.tile_pool(name="b3_w", bufs=1) as b3_w,
                    tc.tile_pool(name="b3_s", bufs=3) as b3_s,
                    tc.tile_pool(name="b3_sc", bufs=1) as b3_sc,
                    tc.tile_pool(name="b3_ps", bufs=1, space="PSUM") as b3_ps,
                    tc.tile_pool(name="b3_bc", bufs=2, space="PSUM") as b3_bc,
                ):
                    wkb = []
                    for k in range(NKV):
                        t_ = b3_w.tile([128, KB_N], B16, name=f"wkb{k}", tag=f"wkb{k}")
                        nc.gpsimd.dma_start(out=t_[:], in_=wkvb[k * 128:(k + 1) * 128, :])
                        wkb.append(t_)
                    # hi/lo rms-scale rows for all ranks, loaded once
                    skvh_all = b3_sc.tile([1, T], B16, name="skvh", tag="skvh")
                    skvl_all = b3_sc.tile([1, T], B16, name="skvl", tag="skvl")
                    for r in range(NCORES):
                        nc.scalar.dma_start(
                            out=skvh_all[0:1, r * TSH:(r + 1) * TSH],
                            in_=ag_kv_out[r * KVB2 + KVB:r * KVB2 + KVB + 1, :])
                        nc.sync.dma_start(
                            out=skvl_all[0:1, r * TSH:(r + 1) * TSH],
                            in_=ag_kv_out[r * KVB2 + KVB + 1:r * KVB2 + KVB2, :])

                    for nb in range(NB):
                        cs = slice(nb * TB, (nb + 1) * TB)
                        skv = b3_s.tile([1, TB], R, name="skv", tag="skv")
                        with nc.allow_low_precision(reason="fp32r rms scale for PE broadcast"):
                            nc.vector.tensor_add(skv[:], skvh_all[0:1, cs], skvl_all[0:1, cs])
                        skvb = b3_bc.tile([128, TB], F, name="skvb", tag="skvb")
                        nc.tensor.matmul(skvb[:], ones_rf[:],
                                         skv[:], start=True, stop=True)
                        kva = []
                        for k in range(NKV):
                            t_ = b3_s.tile([128, TB], B16, name=f"kva{k}", tag=f"kva{k}")
                            for half in range(2):
                                r = 2 * nb + half
                                eng = (nc.sync, nc.scalar, nc.gpsimd)[(2 * k + half) % 3]
                                eng.dma_start(
                                    out=t_[:, half * TSH:(half + 1) * TSH],
                                    in_=ag_kv_out[r * KVB2 + k * 128:r * KVB2 + (k + 1) * 128, :])
                            nc.vector.tensor_mul(t_[:], t_[:], skvb[:])
                            kva.append(t_)
                        psk = [b3_ps.tile([128, TB], F, name=f"b3k{m}", tag=f"b3k{m}") for m in range(HL)]
                        for k in range(NKV):
                            for m in range(HL):
                                nc.tensor.matmul(
                                    psk[m][:], wkb[k][:, m * 128:(m + 1) * 128], kva[k][:],
                                    start=(k == 0), stop=(k == NKV - 1))
                        # v chains use the broadcast pool's banks so the
                        # k_nope drains overlap the v matmuls
                        psvs = []
                        for tch in range(4):
                            psv = b3_bc.tile([128, TB], F, name="b3v", tag="b3v")
                            for k in range(NKV):
                                nc.tensor.matmul(
                                    psv[:], kva[k][:, tch * 128:(tch + 1) * 128],
                                    wkb[k][:, HL * D_NOPE:KB_N],
                                    start=(k == 0), stop=(k == NKV - 1))
                            psvs.append(psv)
                            if tch == 0:
                                for m in range(HL):
                                    if m % 2 == 0:
                                        nc.vector.tensor_copy(kn[m][:, nb * TB:(nb + 1) * TB], psk[m][:])
                                    else:
                                        nc.scalar.copy(out=kn[m][:, nb * TB:(nb + 1) * TB], in_=psk[m][:])
                            if tch >= 1:
                                psd = psvs[tch - 1]
                                nc.vector.tensor_copy(vt[nb * 4 + tch - 1][:], psd[:])
                        nc.vector.tensor_copy(vt[nb * 4 + 3][:], psvs[3][:])

                # ============================ Stage B1/B2: q projection + rope
                with tc.tile_pool(name="b_q", bufs=1) as b_q:
                    qn = [b_q.tile([128, T], B16, name=f"qn{h}", tag=f"qn{h}") for h in range(HL)]
                    qpe_fin = [b_q.tile([128, T], B16, name=f"qpf{t_}", tag=f"qpf{t_}") for t_ in range(2)]

                    with (
                        tc.tile_pool(name="b1_w", bufs=1) as b1_w,
                        tc.tile_pool(name="b1_s", bufs=4) as b1_s,
                        tc.tile_pool(name="b1_sc", bufs=1) as b1_sc,
                        tc.tile_pool(name="b2_t", bufs=2) as b2_t,
                        tc.tile_pool(name="b1_ps", bufs=1, space="PSUM") as b1_ps,
                        tc.tile_pool(name="b2_ps", bufs=2, space="PSUM") as b2_ps,
                    ):
                        wq = []
                        for k in range(NKQ):
                            t_ = b1_w.tile([128, QB_N], B16, name=f"wq{k}", tag=f"wq{k}")
                            eng = (nc.scalar, nc.gpsimd)[k % 2]
                            eng.dma_start(out=t_[:], in_=wqb[k * 128:(k + 1) * 128, :])
                            wq.append(t_)
                        psw_sb = b1_w.tile([128, 128], B16, name="psw", tag="psw")
                        nc.sync.dma_start(out=psw_sb[:], in_=psw[:])
                        # hi/lo rms-scale rows for all ranks, loaded once
                        srh = b1_sc.tile([1, T], B16, name="srh", tag="srh")
                        srl = b1_sc.tile([1, T], B16, name="srl", tag="srl")
                        for r in range(NCORES):
                            nc.scalar.dma_start(
                                out=srh[0:1, r * TSH:(r + 1) * TSH],
                                in_=ag_qa_out[1][r * QAH2 + QAH:r * QAH2 + QAH + 1, :])
                            nc.sync.dma_start(
                                out=srl[0:1, r * TSH:(r + 1) * TSH],
                                in_=ag_qa_out[1][r * QAH2 + QAH + 1:r * QAH2 + QAH2, :])
                        # all 4 token blocks' scale broadcasts up front (gpsimd)
                        rqbs = []
                        for nb in range(NB):
                            cs = slice(nb * TB, (nb + 1) * TB)
                            sqv = b1_sc.tile([1, TB], F, name=f"sqv{nb}", tag=f"sqv{nb}")
                            nc.vector.tensor_add(sqv[:], srh[0:1, cs], srl[0:1, cs])
                            rqb = b1_sc.tile([128, TB], F, name=f"rqb{nb}", tag=f"rqb{nb}")
                            nc.gpsimd.partition_broadcast(rqb[:], sqv[:])
                            rqbs.append(rqb)

                        for nb in range(NB):
                            cs = slice(nb * TB, (nb + 1) * TB)
                            rqb = rqbs[nb]
                            ps6 = [b1_ps.tile([128, TB], F, name=f"b1p{m}", tag=f"b1p{m}") for m in range(6)]
                            for k in range(NKQ):
                                g, kk = divmod(k, 6)
                                stride = QAH + 2 * g
                                rqa = b1_s.tile([128, TB], B16, name="rqa", tag="rqa")
                                for half in range(2):
                                    r = 2 * nb + half
                                    eng = (nc.sync, nc.scalar, nc.gpsimd)[(2 * k + half) % 3]
                                    eng.dma_start(
                                        out=rqa[:, half * TSH:(half + 1) * TSH],
                                        in_=ag_qa_out[g][r * stride + kk * 128:r * stride + (kk + 1) * 128, :])
                                for m in range(6):
                                    nc.tensor.matmul(
                                        ps6[m][:], wq[k][:, m * 128:(m + 1) * 128], rqa[:],
                                        start=(k == 0), stop=(k == NKQ - 1))
                            for m in range(HL):
                                nc.vector.tensor_mul(qn[m][:, cs], ps6[m][:], rqb[:])
                            # rope in place on the scaled q_pe; cos/sin tiles
                            # shared by both q_pe tiles of this token block
                            ccs = b2_t.tile([128, TB], F, name="ccs", tag="ccs")
                            sss = b2_t.tile([128, TB], F, name="sss", tag="sss")
                            nc.scalar.dma_start(out=ccs[:], in_=ccq[:, cs])
                            nc.scalar.dma_start(out=sss[:], in_=ssq[:, cs])
                            for t_ in range(2):
                                raw = ps6[4 + t_]
                                nc.vector.tensor_mul(qpe_fin[t_][:, cs], raw[:], rqb[:])
                                ps_sw = b2_ps.tile([128, TB], F, name="sw", tag="sw")
                                nc.tensor.matmul(ps_sw[:], psw_sb[:], qpe_fin[t_][:, cs],
                                                 start=True, stop=True)
                                tm1 = b2_t.tile([128, TB], F, name="tm1", tag="tm1")
                                tm2 = b2_t.tile([128, TB], F, name="tm2", tag="tm2")
                                nc.vector.tensor_mul(tm1[:], qpe_fin[t_][:, cs], ccs[:])
                                nc.vector.tensor_mul(tm2[:], ps_sw[:], sss[:])
                                nc.vector.tensor_add(qpe_fin[t_][:, cs], tm1[:], tm2[:])

                    # qsw: qpe_fin with its 64-row halves swapped (SBUF-to-SBUF
                    # partition-shift DMAs) so every head's q_pe exists at both
                    # partition offsets; paired kpe score matmuls then run on
                    # disjoint PE row-groups (concurrent in the array).
                    qsw = [b_q.tile([128, T], B16, name=f"qsw{t_}", tag=f"qsw{t_}")
                           for t_ in range(2)]
                    for t_ in range(2):
                        nc.sync.dma_start(out=qsw[t_][0:64, :], in_=qpe_fin[t_][64:128, :])
                        nc.scalar.dma_start(out=qsw[t_][64:128, :], in_=qpe_fin[t_][0:64, :])

                    import contextlib
                    cstack = contextlib.ExitStack()
                    c_w = cstack.enter_context(tc.tile_pool(name="c_w", bufs=2))
                    c_r = cstack.enter_context(tc.tile_pool(name="c_r", bufs=2))
                    c_pref = {}

                    # ============== Stage B4: causal attention, per head
                    with (
                        tc.tile_pool(name="b4_c", bufs=1) as b4_c,
                        tc.tile_pool(name="b4_at", bufs=1) as b4_at,
                        tc.tile_pool(name="b4_e", bufs=3) as b4_e,
                        tc.tile_pool(name="b4_sm", bufs=2) as b4_sm,
                        tc.tile_pool(name="b4_ps", bufs=2, space="PSUM") as b4_ps,
                        tc.tile_pool(name="b4_po", bufs=2, space="PSUM") as b4_po,
                        tc.tile_pool(name="b4_dn", bufs=2, space="PSUM") as b4_dn,
                    ):
                        mask_sb = b4_c.tile([128, 4 * 512], B16, name="mask", tag="mask")
                        nc.sync.dma_start(out=mask_sb[:], in_=maskd[:])
                        at = [b4_at.tile([D_V, T], B16, name=f"at{h}", tag=f"at{h}")
                              for h in range(HL)]

                        def _outden(st):
                            h_, qj_, g_, nki_, ex_, ps_o_, ps_dn_ = st
                            for ci in range(2):
                                ki = 2 * g_ + ci
                                nc.tensor.matmul(
                                    ps_o_[:], vt[ki][:, h_ * 128:(h_ + 1) * 128],
                                    ex_[:, ci * TB:(ci + 1) * TB],
                                    start=(ki == 0), stop=(ki == nki_ - 1))
                            for ci in range(2):
                                ki = 2 * g_ + ci
                                nc.tensor.matmul(
                                    ps_dn_[0:1, :], ones_sb[:, 0:1],
                                    ex_[:, ci * TB:(ci + 1) * TB],
                                    start=(ki == 0), stop=(ki == nki_ - 1))

                        def _norm_a(pn):
                            # invert the softmax denominator row; the result is
                            # consumed by _norm_b two score-groups later so the
                            # PE never waits on the reciprocal
                            h_, qj_, ps_o_, ps_dn_ = pn
                            recb1r = b4_sm.tile([1, TB], R, name="recb1r", tag="recb1r")
                            with nc.allow_low_precision(reason="fp32r softmax denom for PE broadcast"):
                                nc.vector.reciprocal(out=recb1r[:], in_=ps_dn_[0:1, :])
                            return recb1r

                        def _norm_b(entry):
                            # broadcast 1/den with a ones-row matmul, then
                            # normalize + ship this (h, qj)
                            _, pn, recb1r = entry
                            h_, qj_, ps_o_, ps_dn_ = pn
                            qs_ = slice(qj_ * TB, (qj_ + 1) * TB)
                            nc.tensor.matmul(ps_dn_[:], ones_rf[:], recb1r[:],
                                             start=True, stop=True)
                            recb_sb = b4_sm.tile([128, TB], F, name="recbb", tag="recbb")
                            nc.vector.tensor_copy(recb_sb[:], ps_dn_[:])
                            nc.vector.tensor_mul(at[h_][:, qs_], ps_o_[:], recb_sb[:])
                            nc.sync.dma_start(out=ag2_in[h_][:, qs_], in_=at[h_][:, qs_])
                            if qj_ == NB - 1:
                                nc.gpsimd.collective_compute(
                                    "AllGather", mybir.AluOpType.bypass, replica_groups=RG,
                                    ins=[ag2_in[h_].opt()], outs=[ag2_out[h_].opt()])
                                if h_ == 0:
                                    # prefetch stage C's slot-0 weights and
                                    # gathered activations while B4 continues
                                    pw, pr = [], []
                                    for r_ in range(NCORES):
                                        t_ = c_w.tile([128, WO_N], B16,
                                                      name=f"wos{r_}", tag=f"wos{r_}")
                                        nc.scalar.dma_start(
                                            out=t_[:], in_=wo[r_ * 128:(r_ + 1) * 128, :])
                                        pw.append(t_)
                                        t_ = c_r.tile([128, T], B16,
                                                      name=f"rat{r_}", tag=f"rat{r_}")
                                        nc.gpsimd.dma_start(
                                            out=t_[:], in_=ag2_out[0][r_ * 128:(r_ + 1) * 128, :])
                                        pr.append(t_)
                                    c_pref["wos"] = pw
                                    c_pref["rats"] = pr

                        pend_od = None   # score group awaiting out/den matmuls
                        norm_q = []      # [slots-left, (h, qj, ps_o, ps_dn), recb1r]
                        for h in range(HL):
                            t0 = h // 2
                            # this head's q_pe at both partition offsets
                            qpe_b0 = (qpe_fin[t0] if h % 2 == 0 else qsw[t0])
                            qpe_b64 = (qsw[t0] if h % 2 == 0 else qpe_fin[t0])
                            for qj in range(NB):
                                qs = slice(qj * TB, (qj + 1) * TB)
                                nki = 4 * qj + 4
                                ngrp = nki // 2
                                ps_o = b4_po.tile([128, TB], F, name="pso", tag="pso")
                                ps_dn = b4_dn.tile([128, TB], F, name="psdn", tag="psdn")
                                for g in range(ngrp):
                                    ps_s = b4_ps.tile([128, 2 * TB], F, name="pss", tag="pss")
                                    ks0 = slice(2 * g * 128, (2 * g + 1) * 128)
                                    ks1 = slice((2 * g + 1) * 128, (2 * g + 2) * 128)
                                    nc.tensor.matmul(
                                        ps_s[:, 0:TB], kn[h][:, ks0], qn[h][:, qs],
                                        start=True, stop=False)
                                    nc.tensor.matmul(
                                        ps_s[:, TB:2 * TB], kn[h][:, ks1], qn[h][:, qs],
                                        start=True, stop=False)
                                    # the two 64-deep rope matmuls sit on
                                    # disjoint PE row-groups -> run concurrently
                                    nc.tensor.matmul(
                                        ps_s[:, 0:TB], kpe_f[0:64, ks0],
                                        qpe_b0[0:64, qs], start=False, stop=True)
                                    nc.tensor.matmul(
                                        ps_s[:, TB:2 * TB], kpe_f[64:128, ks1],
                                        qpe_b64[64:128, qs], start=False, stop=True)
                                    ex = b4_e.tile([128, 2 * TB], B16, name="ex", tag="ex")
                                    nc.scalar.activation(out=ex[:], in_=ps_s[:], func=AF.Exp)
                                    if g >= ngrp - 2:
                                        d0 = 2 * (g - (ngrp - 2))
                                        nc.vector.tensor_mul(
                                            ex[:], ex[:], mask_sb[:, d0 * TB:(d0 + 2) * TB])
                                    for e_ in norm_q:
                                        e_[0] -= 1
                                    if norm_q and norm_q[0][0] <= 0:
                                        _norm_b(norm_q.pop(0))
                                    if pend_od is not None:
                                        _outden(pend_od)
                                        if pend_od[2] == pend_od[3] // 2 - 1:
                                            pn_ = (pend_od[0], pend_od[1],
                                                   pend_od[5], pend_od[6])
                                            norm_q.append([2, pn_, _norm_a(pn_)])
                                        pend_od = None
                                    pend_od = (h, qj, g, nki, ex, ps_o, ps_dn)
                        # flush the software pipeline
                        _outden(pend_od)
                        pn_ = (pend_od[0], pend_od[1], pend_od[5], pend_od[6])
                        norm_q.append([0, pn_, _norm_a(pn_)])
                        while norm_q:
                            _norm_b(norm_q.pop(0))

                    # ============== Stage C: output projection (dense phase)
                    with (
                        tc.tile_pool(name="c_acc", bufs=1) as c_acc,
                        tc.tile_pool(name="c_ps", bufs=4, space="PSUM") as c_ps,
                    ):
                        acc = [c_acc.tile([128, TB], F, name=f"acc{i}", tag=f"acc{i}")
                               for i in range(16)]
                        for j in range(HL):
                            if j == 0:
                                wos = c_pref["wos"]
                                rats = c_pref["rats"]
                            else:
                                wos = []
                                for r in range(NCORES):
                                    t_ = c_w.tile([128, WO_N], B16, name=f"wos{r}", tag=f"wos{r}")
                                    nc.scalar.dma_start(
                                        out=t_[:],
                                        in_=wo[(j * NCORES + r) * 128:(j * NCORES + r + 1) * 128, :])
                                    wos.append(t_)
                                rats = []
                                for r in range(NCORES):
                                    t_ = c_r.tile([128, T], B16, name=f"rat{r}", tag=f"rat{r}")
                                    eng = (nc.sync, nc.gpsimd)[r % 2]
                                    eng.dma_start(
                                        out=t_[:], in_=ag2_out[j][r * 128:(r + 1) * 128, :])
                                    rats.append(t_)
                            for mo in range(4):
                                for nb in range(NB):
                                    psc = c_ps.tile([128, TB], F, name="psc", tag="psc")
                                    for r in range(NCORES):
                                        nc.tensor.matmul(
                                            psc[:],
                                            wos[r][:, mo * 128:(mo + 1) * 128],
                                            rats[r][:, nb * TB:(nb + 1) * TB],
                                            start=(r == 0), stop=(r == NCORES - 1))
                                    a_ = acc[mo * 4 + nb]
                                    if j == 0:
                                        nc.scalar.copy(out=a_[:], in_=psc[:])
                                    elif j < HL - 1:
                                        nc.vector.tensor_add(a_[:], a_[:], psc[:])
                                    else:
                                        nc.vector.tensor_add(a_[:], a_[:], psc[:])
                                        eng = (nc.sync, nc.gpsimd)[(mo * 4 + nb) % 2]
                                        eng.dma_start(
                                            out=out[mo * 128:(mo + 1) * 128,
                                                    nb * TB:(nb + 1) * TB],
                                            in_=a_[:])
                    cstack.close()

    nc.compile()
    _CACHE["nc"] = nc
    return nc


# ---------------------------------------------------------------- host prep
def _prep_inputs(positions, hidden_states, Wqa, q_a_ln, Wqb, Wkva, kv_a_ln, Wkvb, Wo):
    import ml_dtypes

    positions = np.asarray(positions)
    hidden_states = np.ascontiguousarray(np.asarray(hidden_states, dtype=np.float32))
    Wqa = np.ascontiguousarray(np.asarray(Wqa, dtype=np.float32))
    q_a_ln = np.asarray(q_a_ln, dtype=np.float32)
    Wqb = np.asarray(Wqb, dtype=np.float32)
    Wkva = np.asarray(Wkva, dtype=np.float32)
    kv_a_ln = np.asarray(kv_a_ln, dtype=np.float32)
    Wkvb = np.asarray(Wkvb, dtype=np.float32)
    Wo = np.ascontiguousarray(np.asarray(Wo, dtype=np.float32))

    mscale = 0.1 * MSCALE_ALL_DIM * math.log(FACTOR) + 1.0
    scaling = (D_QK ** -0.5) * mscale * mscale

    inv_freq = _yarn_inv_freq()
    freqs = positions.astype(np.float32)[:, None] * inv_freq[None, :]  # [T, 32]
    cos = np.cos(freqs).astype(np.float32)
    sin = np.sin(freqs).astype(np.float32)

    HR = D_ROPE // 2
    perm = np.concatenate([np.arange(0, D_ROPE, 2), np.arange(1, D_ROPE, 2)])  # even|odd

    # Wqb: fold q_a_ln + scaling, permute per-core columns
    wqb_eff = (q_a_ln[:, None] * Wqb).reshape(QA, H, D_QK) * scaling
    wqb_cores = []
    for c in range(NCORES):
        hs = range(c * HL, (c + 1) * HL)
        cols = [wqb_eff[:, h_, :D_NOPE] for h_ in hs]
        cols += [wqb_eff[:, h_, D_NOPE + perm] for h_ in hs]
        wqb_cores.append(np.ascontiguousarray(
            np.concatenate(cols, axis=1).astype(ml_dtypes.bfloat16)))

    # Wkva: rope perm on the k_pe columns
    wkva_p = Wkva.copy()
    wkva_p[:, KV_LORA:] = Wkva[:, KV_LORA + perm]
    wkva_p = np.ascontiguousarray(wkva_p, dtype=np.float32)
    wkva_b = np.ascontiguousarray(wkva_p.astype(ml_dtypes.bfloat16))
    wqa_b = np.ascontiguousarray(Wqa.astype(ml_dtypes.bfloat16))

    # Wkvb: fold kv_a_ln, per-core [k_nope x4 | v x4]
    wkvb_eff = (kv_a_ln[:, None] * Wkvb).reshape(KV_LORA, H, D_NOPE + D_V)
    wkvb_cores = []
    for c in range(NCORES):
        hs = range(c * HL, (c + 1) * HL)
        cols = [wkvb_eff[:, h_, :D_NOPE] for h_ in hs]
        cols += [wkvb_eff[:, h_, D_NOPE:] for h_ in hs]
        wkvb_cores.append(np.ascontiguousarray(
            np.concatenate(cols, axis=1).astype(ml_dtypes.bfloat16)))

    # Wo rows permuted to the stage-C gather order: slot j, rank r -> head 4r+j
    row_order = []
    for j in range(HL):
        for r in range(NCORES):
            h_ = HL * r + j
            row_order.extend(range(h_ * D_V, (h_ + 1) * D_V))
    wo_p = Wo[row_order, :].astype(ml_dtypes.bfloat16)

    # rope ext tiles for q (2 heads per 128-row tile: [e,o | e,o] x 32 rows each)
    cosT = cos.T  # [32, T]
    sinT = sin.T
    ccq = np.ascontiguousarray(np.tile(cosT, (4, 1)), dtype=np.float32)      # [128, T]
    ssq = np.ascontiguousarray(np.concatenate([-sinT, sinT, -sinT, sinT], axis=0), dtype=np.float32)

    # swap permutation: within each 64-row block, rows 0:32 <-> 32:64
    pswm = np.zeros((128, 128), dtype=np.float32)
    for j in range(128):
        base = (j // 64) * 64
        off = j % 64
        k = base + (off + HR) % 64
        pswm[k, j] = 1.0
    pswm = pswm.astype(ml_dtypes.bfloat16)

    # causal masks for the 4 diagonal offsets (512-wide q blocks, 128-wide k chunks)
    pos = positions.astype(np.int64)
    maskd = np.zeros((128, 4 * 512), dtype=np.float32)
    for d in range(4):
        kpos = pos[d * 128:(d + 1) * 128]   # relative within a q block
        qpos = pos[0:512]
        maskd[:, d * 512:(d + 1) * 512] = (kpos[:, None] <= qpos[None, :]).astype(np.float32)
    maskd = maskd.astype(ml_dtypes.bfloat16)

    per_core = []
    for c in range(NCORES):
        sl = slice(c * TSH, (c + 1) * TSH)
        hT_c = hidden_states[sl].T.astype(ml_dtypes.bfloat16)   # [HID, TSH]
        per_core.append({
            "wA": np.ascontiguousarray(
                np.concatenate([wkva_b, hT_c, wqa_b], axis=1)),
            "wqb": wqb_cores[c],
            "wkvb": wkvb_cores[c],
            "wo": np.ascontiguousarray(wo_p[:, c * WO_N:(c + 1) * WO_N]),
            "cca": np.ascontiguousarray(cosT[:, sl]),
            "ssa": np.ascontiguousarray(sinT[:, sl]),
            "ccq": ccq,
            "ssq": ssq,
            "psw": pswm,
            "maskd": maskd,
            "onesd": np.ones((128, 128), dtype=ml_dtypes.bfloat16),
        })
    return per_core


def run(inputs, trace=False):
    """Build + run; returns (full_output [T, HID] fp32, exec_time_ns or None)."""
    _install_ntff_hook()
    from concourse.bass_utils import run_bass_kernel_spmd

    nc = _build_program()
    in_maps = _prep_inputs(**inputs)
    res = run_bass_kernel_spmd(nc, in_maps, list(range(NCORES)), trace=trace)
    out = np.empty((T, HID), dtype=np.float32)
    for c in range(NCORES):
        out[:, c * WO_N:(c + 1) * WO_N] = res.results[c]["out"].T
    return out, res.exec_time_ns


def kernel(**inputs):
    out, _ = run(inputs, trace=False)
    return out
